# revision 1
# baseline (speedup 1.0000x reference)
"""CombinedSurvLoss (NLL survival + pairwise ranking) on 8 TRN2 NeuronCores.

Math
----
reference = mean_j L_j + 0.1 * total / count, where

  L_j     = -(1-c_j) * ln(clip(s_prev_j) * clip(h_j)) - 0.85 * c_j * ln(clip(s_now_j))
  total   = sum_{i,j} [c_i=0][Y_j>Y_i] relu(r_j - r_i),  r = hazards.sum(axis=1)
  count   = sum_{i,j} [c_i=0][Y_j>Y_i]

The O(B^2) ranking term is decomposed through per-class weight vectors
(K=4 classes):  p^a_i = [Y_i=a][c_i=0],  q^b_i = [Y_i=b].  With
V^{(u)}_x = sum_i u_i [r_x > r_i] (8 weighted-rank vectors sharing one
comparison matrix), one shows

  total = sum_x r_x * ( sum_{a<Y_x} V^{(p^a)}_x
                        - [c_x=0] * sum_{b>Y_x} (Q_b - V^{(q^b)}_x) )
  count = sum_{a<b} P_a Q_b          (P_a = sum p^a, Q_b = sum q^b)

On device the comparison tile C[i, x] = [r_x > r_i] is produced by one pass
per 128-row i-block — spread across DVE (is_gt, bf16), ACT (Sign into fp8,
with halved weights plus a correction K precomputed from U), and Pool
(is_gt, fp8) — and immediately contracted against the 8 weight columns on
the TensorEngine (PSUM-accumulated U^T C; plain bf16 matmuls for DVE blocks,
fp8 DoubleRow pairs for the rest), so the B^2 work runs at matmul rate
instead of 3-4 vector passes per element.  Each core
owns a 1024-column x-slice and loops over all 64 i-blocks, so V is complete
per-core without a collective; the final scalar partial is AllReduce-summed
on device.

Sharding: x-slice (batch dim) of hazards/S/Y/c per core for the per-row
work; the full hazards/Y/c (393KB) are also DMA'd to every core so each
core can form all 64 i-blocks of the pairwise matrix (the "all-gather of
risk/Y/c" from the sharding hint, done host-side as input replication).
"""

import numpy as np

import concourse.mybir as mybir
import concourse.tile as tile
from concourse import bacc
from concourse.bass_utils import run_bass_kernel_spmd
from concourse.masks import make_identity

F32 = mybir.dt.float32
BF16 = mybir.dt.bfloat16
F8 = mybir.dt.float8e4
I32 = mybir.dt.int32
AF = mybir.ActivationFunctionType
ALU = mybir.AluOpType
AX = mybir.AxisListType

NCORES = 8
B, K = 8192, 4
SH = B // NCORES          # 1024 rows per core
NBLK = B // 128           # 64 i-blocks (all rows, every core)
OWN = SH // 128           # 8 j-chunks of the core's own slice
CW = SH                  # compare width: the core's 1024 own columns
ALPHA = 0.15
RANKING_WEIGHT = 0.1
EPS = 1e-7

# Compare-work schedule. Each i-block's compare tile is produced by one of:
#   bf16 C + plain bf16 matmul        on DVE (fast 16-bit DVE mode, PE has slack)
#   fp8  C2 pair + DoubleRow matmul   on ACT (Sign), Pool (is_gt) or DVE (is_gt)
# Items are interleaved so the in-order PSUM accumulation chain never waits on
# a run of slow-engine tiles; block indices are assigned in schedule order
# (any i->block permutation is valid), which keeps fp8 pairs index-adjacent.
N_BF = 30            # DVE bf16 blocks
_F8_QUOTA = {"A": 16, "P": 10, "D": 8}   # fp8 blocks per engine (all even)
N_ACT = _F8_QUOTA["A"]


def _interleave(quota):
    acc = dict.fromkeys(quota, 0.0)
    total = sum(quota.values())
    out = []
    for _ in range(total):
        for k in quota:
            acc[k] += quota[k] / total
        pick = max(acc, key=lambda k: acc[k])
        acc[pick] -= 1.0
        out.append(pick)
    return out


def _mk_schedule():
    # block-index ranges: [0, A) ACT pairs, [A, A+P) Pool pairs, [A+P, A+P+D)
    # DVE-fp8 pairs, rest bf16 — ACT blocks contiguous at the front so the
    # Sign-halved weight correction K is a simple strided reduce over U[:,0:16]
    nxt = {}
    base = 0
    for k in ("A", "P", "D"):
        nxt[k] = base
        base += _F8_QUOTA[k]
    nxt["bf"] = base
    pair_engines = _interleave(
        {k: v // 2 for k, v in _F8_QUOTA.items()}
    )  # 17 pairs
    kinds = _interleave({"bf": N_BF, "f8": len(pair_engines)})
    sched = []
    pi = iter(pair_engines)
    for kind in kinds:
        if kind == "bf":
            sched.append(("bf", nxt["bf"], None))
            nxt["bf"] += 1
        else:
            e = next(pi)
            sched.append(("f8", nxt[e], (e, e)))
            nxt[e] += 2
    return sched


SCHEDULE = _mk_schedule()
ACT_BLOCKS = frozenset(range(N_ACT))

DO_COLLECTIVE = True


def _build_program():
    nc = bacc.Bacc(
        "TRN2",
        target_bir_lowering=False,
        debug=False,
        enable_asserts=False,
        num_devices=NCORES,
    )

    hz_full = nc.dram_tensor("hz_full", [B, K], F32, kind="ExternalInput").ap()
    yc_full = nc.dram_tensor("yc_full", [2, B], I32, kind="ExternalInput").ap()
    hzT_own = nc.dram_tensor("hzT_own", [K, SH], F32, kind="ExternalInput").ap()
    hs_own = nc.dram_tensor("hs_own", [2, SH, K], F32, kind="ExternalInput").ap()
    yc_own = nc.dram_tensor("yc_own", [2, SH], I32, kind="ExternalInput").ap()
    out = nc.dram_tensor("out", [1, 1], F32, kind="ExternalOutput").ap()

    with tile.TileContext(nc) as tc:
        with (
            tc.tile_pool(name="const", bufs=1) as constp,
            tc.tile_pool(name="sb", bufs=1) as sb,
            tc.tile_pool(name="cmp", bufs=12) as cmpp,
            tc.tile_pool(name="ps", bufs=1, space="PSUM") as ps,
            tc.tile_pool(name="pst", bufs=1, space="PSUM") as pst,
            tc.tile_pool(name="psrb", bufs=2, space="PSUM") as psrb,
        ):
            # ---------- constants (only ones4 up front: the rb chain that
            # gates the first compare needs it; the rest come after) ----------
            ones4 = constp.tile([4, 128], BF16)
            nc.vector.memset(ones4[:], 1.0)

            # ---------- input loads (hzT first: it heads the rb chain that
            # gates the first compare; one DMA per packed host tensor) ----------
            hzT = sb.tile([4, SH], F32)
            nc.sync.dma_start(hzT[:], hzT_own)
            hzT_bf = sb.tile([4, SH], BF16)
            nc.vector.tensor_copy(hzT_bf[:], hzT[:])
            # full hazards [p, blk, k] with global row i = p*NBLK + blk
            # (p-major keeps each partition's DMA read contiguous; any i->(p,blk)
            # assignment works since the pairwise sum runs over all i)
            hzp = sb.tile([128, NBLK, K], F32)
            hz_re = hz_full.rearrange("(p b) k -> p b k", p=128)
            nc.sync.dma_start(hzp[0:64], hz_re[0:64])
            nc.gpsimd.dma_start(hzp[64:128], hz_re[64:128])
            # full Y / c in the same [p, blk] layout
            yc = sb.tile([128, 2, NBLK], I32)
            nc.sync.dma_start(yc[:], yc_full.rearrange("t (p b) -> p t b", p=128))
            yi, ci = yc[:, 0, :], yc[:, 1, :]
            # own-slice per-row tiles [p, jc, k], own row j = jc*128 + p
            hso = sb.tile([128, 2, OWN, K], F32)
            nc.sync.dma_start(hso[:], hs_own.rearrange("t (b p) k -> p t b k", p=128))
            hzo, so = hso[:, 0], hso[:, 1]
            yco = sb.tile([128, 2, OWN], I32)
            nc.sync.dma_start(yco[:], yc_own.rearrange("t (b p) -> p t b", p=128))
            yoi, coi = yco[:, 0, :], yco[:, 1, :]

            # ---------- derived setup ----------
            # rb[p, x] = r_x broadcast to all partitions (PE ones-matmul, bf16)
            rb = sb.tile([128, CW], BF16)
            for ch in range(2):
                sl = slice(ch * 512, (ch + 1) * 512)
                ps_rb = psrb.tile([128, 512], F32, tag="rb")
                nc.tensor.matmul(
                    ps_rb[:], lhsT=ones4[:], rhs=hzT_bf[:, sl], start=True, stop=True
                )
                nc.vector.tensor_copy(rb[:, sl], ps_rb[:])

            ones1 = constp.tile([1, 128], F32)
            nc.vector.memset(ones1[:], 1.0)
            onescol = constp.tile([128, 1], F32)
            nc.vector.memset(onescol[:], 1.0)
            id8 = constp.tile([8, 8], F32)
            make_identity(nc, id8[:])
            id16 = constp.tile([16, 16], F32)
            make_identity(nc, id16[:])

            # r_all[p, blk] = full risk, used as per-i-block compare scalars
            r_all = sb.tile([128, NBLK], F32)
            nc.vector.tensor_reduce(r_all[:], hzp[:], axis=AX.X, op=ALU.add)
            neg_r = sb.tile([128, NBLK], F32)
            nc.vector.tensor_scalar(
                neg_r[:], r_all[:], -1.0, None, op0=ALU.mult
            )

            yf = sb.tile([128, NBLK], F32)
            nc.vector.tensor_copy(yf[:], yi)
            cbar = sb.tile([128, NBLK], F32)  # 1 - c  (uncensored indicator)
            nc.vector.tensor_scalar(
                cbar[:], ci, -1.0, 1.0, op0=ALU.mult, op1=ALU.add
            )

            # weight matrix U[p, blk, u]: u 0..3 = p^a, u 4..7 = q^a; built
            # in bf16 (used directly by the bf16 matmuls), then copied to an
            # fp8 version padded to 16 u-cols (16-aligned DoubleRow Ko step)
            Ubf = sb.tile([128, NBLK, 8], BF16)
            tmp_eq = sb.tile([128, NBLK], F32)
            for a in range(4):
                nc.vector.tensor_scalar(
                    Ubf[:, :, 4 + a], yf[:], float(a), None, op0=ALU.is_equal
                )
                nc.vector.tensor_scalar(
                    tmp_eq[:], yf[:], float(a), None, op0=ALU.is_equal
                )
                nc.vector.tensor_tensor(
                    Ubf[:, :, a], tmp_eq[:], cbar[:], op=ALU.mult
                )
            U = sb.tile([128, NBLK, 16], F8)
            nc.gpsimd.tensor_copy(U[:, :, 0:8], Ubf[:])
            # halve the weights of the (contiguous) ACT block range: Sign gives
            # {-1,0,1} and sum u_i*(s+1)/2 = sum (u_i/2)*s + K_u
            nc.vector.tensor_scalar(
                U[:, 0:N_ACT, 0:8], U[:, 0:N_ACT, 0:8], 0.5, None, op0=ALU.mult
            )

            # ---------- own-row scalars + NLL (independent of the pairwise
            # loop; emitted early, mostly on Pool, to overlap the main phase)
            yof = sb.tile([128, OWN], F32)
            nc.vector.tensor_copy(yof[:], yoi)
            cobar = sb.tile([128, OWN], F32)  # 1 - c_own
            nc.vector.tensor_scalar(
                cobar[:], coi, -1.0, 1.0, op0=ALU.mult, op1=ALU.add
            )
            ro = sb.tile([128, OWN], F32)
            nc.vector.tensor_reduce(ro[:], hzo, axis=AX.X, op=ALU.add)

            # y-comparison masks for T1/T2, precomputed so the post-loop tail
            # only has the V-dependent multiplies left
            gm = []
            for a in range(3):
                g = sb.tile([128, OWN], F32, tag=f"gm{a}")
                nc.vector.tensor_scalar(g[:], yof[:], float(a), None, op0=ALU.is_gt)
                gm.append(g)
            lm = {}
            for b in range(1, 4):
                l = sb.tile([128, OWN], F32, tag=f"lm{b}")
                nc.vector.tensor_scalar(l[:], yof[:], float(b), None, op0=ALU.is_lt)
                lm[b] = l

            # NLL gather-by-onehot (K=4) on Pool
            e = []
            for k in range(4):
                ek = sb.tile([128, OWN], F32, tag=f"e{k}")
                nc.gpsimd.tensor_scalar(
                    ek[:], yof[:], float(k), None, op0=ALU.is_equal
                )
                e.append(ek)
            acc = sb.tile([128, OWN], F32)

            def gather(dst, src3, shift):
                # dst = sum_k e[k] * src3[:, :, k+shift] (skipping oob)
                first = True
                for k in range(4):
                    kk = k + shift
                    if kk < 0 or kk > 3:
                        continue
                    nc.gpsimd.tensor_tensor(
                        acc[:], e[k][:], src3[:, :, kk], op=ALU.mult
                    )
                    if first:
                        nc.gpsimd.tensor_copy(dst[:], acc[:])
                        first = False
                    else:
                        nc.gpsimd.tensor_tensor(dst[:], dst[:], acc[:], op=ALU.add)

            s_now = sb.tile([128, OWN], F32)
            gather(s_now, so, 0)
            h = sb.tile([128, OWN], F32)
            gather(h, hzo, 0)
            s_prev = sb.tile([128, OWN], F32)
            gather(s_prev, so, -1)  # e1*S0 + e2*S1 + e3*S2
            nc.gpsimd.tensor_tensor(s_prev[:], s_prev[:], e[0][:], op=ALU.add)

            for t in (s_now, h, s_prev):
                nc.gpsimd.tensor_scalar(t[:], t[:], EPS, None, op0=ALU.max)

            # product for ln(s_prev * h); the Ln itself runs in the tail so
            # ACT's function-table sequence during the main loop is Sign-only
            sph = sb.tile([128, OWN], F32)
            nc.gpsimd.tensor_tensor(sph[:], s_prev[:], h[:], op=ALU.mult)

            # ---------- main O(B^2) loop ----------
            # psum_V[u, x] accumulates sum_i u_i * [r_x > r_i] over all i-blocks
            psV = ps.tile([8, CW], F32)
            last = len(SCHEDULE) - 1
            for it, (kind, b0, engs) in enumerate(SCHEDULE):
                if kind == "bf":
                    C1 = cmpp.tile([128, CW], BF16, tag="Cb")
                    nc.vector.tensor_scalar(
                        C1[:], rb[:], r_all[:, b0 : b0 + 1], None, op0=ALU.is_gt
                    )
                    for c0 in range(0, CW, 512):
                        c1 = min(c0 + 512, CW)
                        nc.tensor.matmul(
                            psV[:, c0:c1],
                            lhsT=Ubf[:, b0, :],
                            rhs=C1[:, c0:c1],
                            start=(it == 0),
                            stop=(it == last),
                        )
                else:
                    C2 = cmpp.tile([128, 2, CW], F8, tag="C")
                    for hh in range(2):
                        b = b0 + hh
                        if engs[hh] == "A":
                            nc.scalar.activation(
                                C2[:, hh, :],
                                rb[:],
                                AF.Sign,
                                bias=neg_r[:, b : b + 1],
                            )
                        else:
                            eng = nc.gpsimd if engs[hh] == "P" else nc.vector
                            eng.tensor_scalar(
                                C2[:, hh, :],
                                rb[:],
                                r_all[:, b : b + 1],
                                None,
                                op0=ALU.is_gt,
                            )
                    for c0 in range(0, CW, 512):
                        c1 = min(c0 + 512, CW)
                        nc.tensor.matmul(
                            psV[:, c0:c1],
                            lhsT=U[:, b0 : b0 + 2, 0:8],
                            rhs=C2[:, :, c0:c1],
                            start=(it == 0),
                            stop=(it == last),
                            perf_mode=mybir.MatmulPerfMode.DoubleRow,
                        )

            # ---------- final per-core reduction ----------
            # global per-class sums P/Q and the Sign correction K, straight
            # from U (strided reduces + partition-sum matmul + broadcast);
            # emitted post-loop so they fill the Vs-copy/transpose window
            SS = sb.tile([128, 16], F32)
            for u in range(8):
                nc.vector.tensor_reduce(
                    SS[:, u : u + 1], Ubf[:, :, u], axis=AX.X, op=ALU.add
                )
                nc.vector.tensor_reduce(
                    SS[:, 8 + u : 9 + u], Ubf[:, 0:N_ACT, u], axis=AX.X, op=ALU.add
                )
            ps_ss = pst.tile([16, 1], F32, tag="pq")
            nc.tensor.matmul(
                ps_ss[:], lhsT=SS[:], rhs=onescol[:], start=True, stop=True
            )
            ss_col = sb.tile([16, 1], F32)
            nc.vector.tensor_copy(ss_col[:], ps_ss[:])
            ps_row = pst.tile([1, 16], F32, tag="pqr")
            nc.tensor.transpose(ps_row[:], ss_col[:], id16[:])
            pqk_row = sb.tile([1, 16], F32)
            nc.vector.tensor_copy(pqk_row[:], ps_row[:])
            ps_bc = pst.tile([128, 16], F32, tag="bc")
            nc.tensor.matmul(
                ps_bc[:], lhsT=ones1[:], rhs=pqk_row[:], start=True, stop=True
            )
            QBK = sb.tile([128, 16], F32)  # [:,0:8]=P/Q, [:,8:16]=2K
            nc.vector.tensor_copy(QBK[:], ps_bc[:])
            KQ = sb.tile([128, 8], F32)  # K_u = half the ACT-range sum
            nc.vector.tensor_scalar(
                KQ[:], QBK[:, 8:16], 0.5, None, op0=ALU.mult
            )
            QmK = sb.tile([128, 8], F32)  # Q_u - K_u
            nc.vector.tensor_tensor(
                QmK[:], QBK[:, 0:8], KQ[:], op=ALU.subtract
            )

            # count = sum_{a<b} P_a Q_b -> rscale = 0.1/count (all partitions)
            sfx = sb.tile([128, 3], F32)
            nc.gpsimd.tensor_copy(sfx[:, 2:3], QBK[:, 7:8])
            nc.gpsimd.tensor_tensor(sfx[:, 1:2], QBK[:, 6:7], QBK[:, 7:8], op=ALU.add)
            nc.gpsimd.tensor_tensor(sfx[:, 0:1], QBK[:, 5:6], sfx[:, 1:2], op=ALU.add)
            cnt = sb.tile([128, 3], F32)
            nc.gpsimd.tensor_tensor(cnt[:], QBK[:, 0:3], sfx[:], op=ALU.mult)
            cnt1 = sb.tile([128, 1], F32)
            nc.vector.tensor_reduce(cnt1[:], cnt[:], axis=AX.X, op=ALU.add)
            rscale = sb.tile([128, 1], F32)
            nc.vector.reciprocal(rscale[:], cnt1[:])
            nc.vector.tensor_scalar(
                rscale[:], rscale[:], RANKING_WEIGHT, None, op0=ALU.mult
            )

            # NLL logs first: the Ln table load + [128,8] passes overlap the
            # V transposes below
            Vs = sb.tile([8, CW], F32)
            nc.scalar.copy(Vs[:], psV[:])
            lnsh = sb.tile([128, OWN], F32)
            nc.scalar.activation(lnsh[:], sph[:], AF.Ln)
            lnsn = sb.tile([128, OWN], F32)
            nc.scalar.activation(lnsn[:], s_now[:], AF.Ln)
            # L = -cbar*lnsh - 0.85*lnsn + 0.85*cbar*lnsn
            Lt = sb.tile([128, OWN], F32)
            nc.gpsimd.tensor_tensor(Lt[:], cobar[:], lnsh[:], op=ALU.mult)
            t3 = sb.tile([128, OWN], F32)
            nc.gpsimd.tensor_tensor(t3[:], cobar[:], lnsn[:], op=ALU.mult)
            nc.gpsimd.tensor_scalar(
                t3[:], t3[:], 1.0 - ALPHA, None, op0=ALU.mult
            )
            nc.gpsimd.tensor_tensor(Lt[:], t3[:], Lt[:], op=ALU.subtract)
            nc.gpsimd.tensor_scalar(
                t3[:], lnsn[:], 1.0 - ALPHA, None, op0=ALU.mult
            )
            nc.gpsimd.tensor_tensor(Lt[:], Lt[:], t3[:], op=ALU.subtract)

            # transpose V to [x-partition] layout, all 8 chunks into one PSUM
            # bank: Vt[p, jc*8+u] = V[u, jc*128+p]
            ps_t = pst.tile([128, 64], F32, tag="vt")
            for jc in range(OWN):
                nc.tensor.transpose(
                    ps_t[:, jc * 8 : (jc + 1) * 8],
                    Vs[:, jc * 128 : (jc + 1) * 128],
                    id8[:],
                )
            Vt3 = sb.tile([128, OWN, 8], F32)
            nc.vector.tensor_copy(Vt3[:], ps_t[:])
            Vt = Vt3  # view [128, jc, u]

            # T1 = sum_{a<y} (Vt_a + K_a); T2' = sum_{b>y} (Vt_b - (Q_b - K_b))
            #    = -sum_{b>y} (Q_b - V_true_b); fused (in0 op0 s) op1 in1 ops
            t1p = []
            for a in range(3):
                t = sb.tile([128, OWN], F32, tag=f"t1p{a}")
                nc.vector.scalar_tensor_tensor(
                    t[:], Vt[:, :, a], KQ[:, a : a + 1], gm[a][:],
                    op0=ALU.add, op1=ALU.mult,
                )
                t1p.append(t)
            t2p = []
            for b in range(1, 4):
                t = sb.tile([128, OWN], F32, tag=f"t2p{b}")
                nc.vector.scalar_tensor_tensor(
                    t[:], Vt[:, :, 4 + b], QmK[:, 4 + b : 5 + b], lm[b][:],
                    op0=ALU.subtract, op1=ALU.mult,
                )
                t2p.append(t)
            T1 = sb.tile([128, OWN], F32)
            T2 = sb.tile([128, OWN], F32)
            nc.vector.tensor_tensor(T1[:], t1p[0][:], t1p[1][:], op=ALU.add)
            nc.vector.tensor_tensor(T1[:], T1[:], t1p[2][:], op=ALU.add)
            nc.gpsimd.tensor_tensor(T2[:], t2p[0][:], t2p[1][:], op=ALU.add)
            nc.gpsimd.tensor_tensor(T2[:], T2[:], t2p[2][:], op=ALU.add)

            # contrib = r * (T1 + cbar * T2')   (T2' = -T2_true)
            contrib = sb.tile([128, OWN], F32)
            nc.vector.tensor_tensor(contrib[:], cobar[:], T2[:], op=ALU.mult)
            nc.vector.tensor_tensor(contrib[:], T1[:], contrib[:], op=ALU.add)
            nc.vector.tensor_tensor(contrib[:], contrib[:], ro[:], op=ALU.mult)

            # grand = L/B + contrib * (0.1/count); reduce to a single scalar
            grand = sb.tile([128, OWN], F32)
            nc.vector.tensor_scalar(
                contrib[:], contrib[:], rscale[:, 0:1], None, op0=ALU.mult
            )
            red = sb.tile([128, 1], F32)
            nc.vector.scalar_tensor_tensor(
                grand[:], Lt[:], 1.0 / B, contrib[:],
                op0=ALU.mult, op1=ALU.add, accum_out=red[:],
            )
            ps_fin = pst.tile([1, 1], F32, tag="bc")
            nc.tensor.matmul(
                ps_fin[:], lhsT=red[:], rhs=onescol[:], start=True, stop=True
            )
            partial = sb.tile([1, 1], F32)
            nc.vector.tensor_copy(partial[:], ps_fin[:])

            # ---------- global sum ----------
            if DO_COLLECTIVE:
                with tc.tile_pool(name="dram", bufs=1, space="DRAM") as dramp:
                    cc_in = dramp.tile([1, 1], F32)
                    cc_out = dramp.tile([1, 1], F32)
                    nc.sync.dma_start(cc_in[:], partial[:])
                    nc.gpsimd.collective_compute(
                        "AllReduce",
                        ALU.add,
                        replica_groups=[list(range(NCORES))],
                        ins=[cc_in.opt()],
                        outs=[cc_out.opt()],
                    )
                    nc.sync.dma_start(out[:], cc_out[:])
            else:
                nc.sync.dma_start(out[:], partial[:])

    nc.compile()
    return nc


_PROGRAM = None


def _get_program():
    global _PROGRAM
    if _PROGRAM is None:
        _PROGRAM = _build_program()
    return _PROGRAM


def kernel(hazards, S, Y, c):
    hazards = np.ascontiguousarray(np.asarray(hazards, dtype=np.float32))
    S = np.ascontiguousarray(np.asarray(S, dtype=np.float32))
    Y32 = np.asarray(Y).astype(np.int32)
    c32 = np.asarray(c).astype(np.int32)
    yc_full = np.ascontiguousarray(np.stack([Y32, c32]))

    nc = _get_program()
    in_maps = []
    for m in range(NCORES):
        sl = slice(m * SH, (m + 1) * SH)
        in_maps.append(
            {
                "hz_full": hazards,
                "yc_full": yc_full,
                "hzT_own": np.ascontiguousarray(hazards[sl].T),
                "hs_own": np.ascontiguousarray(
                    np.stack([hazards[sl], S[sl]])
                ),
                "yc_own": np.ascontiguousarray(yc_full[:, sl]),
            }
        )
    res = run_bass_kernel_spmd(nc, in_maps, core_ids=list(range(NCORES)))
    if DO_COLLECTIVE:
        val = res.results[0]["out"][0, 0]
    else:
        val = np.float32(sum(r["out"][0, 0] for r in res.results))
    return np.asarray(val, dtype=np.float32).reshape(())



# revision 10
# speedup vs baseline: 1.4622x; 1.4622x over previous
"""CombinedSurvLoss (NLL survival + pairwise ranking) on 8 TRN2 NeuronCores.

Math
----
reference = mean_j L_j + 0.1 * total / count, where

  L_j     = -(1-c_j) * ln(clip(s_prev_j) * clip(h_j)) - 0.85 * c_j * ln(clip(s_now_j))
  total   = sum_{i,j} [c_i=0][Y_j>Y_i] relu(r_j - r_i),  r = hazards.sum(axis=1)
  count   = sum_{i,j} [c_i=0][Y_j>Y_i]

Binned-rank decomposition of the O(B^2) term
--------------------------------------------
Quantize the (bf16-rounded, canonical) risk r~ into NB uniform bins of
width w over [0, 4).  With per-class indicator weights
p^a_i = [Y_i=a][c_i=0], q^b_i = [Y_i=b] build the per-bin mass table

  H[u, t] = sum_i u_i [t*w <= r~_i < (t+1)*w]        (u over the 8 classes)

via one compare tile per own 128-row chunk (CB[i, e] = [r~_i < (e+1)w],
contracted with U on the TensorEngine -> cumulative F1[u, e], AllReduced
across the 8 cores, then differenced).  A pair (i, j) is counted iff
bin_i < bin_j, consistently on both sides of the decomposition

  total ~= sum_x r_x * ( sum_{a<Y_x} V_A[p^a, x]
                         - [c_x=0] sum_{b>Y_x} (Q_b - V_<=[q^b, x]) )

where V_A[u, x] = sum_t H[u,t][t <  bin_x]  (strict prefix)
      V_<=[u,x] = sum_t H[u,t][t <= bin_x]  (inclusive prefix)

Both gathers share one compare tile set CX[e, x] = [r~_x >= (e+1)w]
(e on partitions) and one PE contraction with shifted weight columns:
cols 0:4 = Hp[e] (A-side), cols 4:8 = Hq[e+1] (inclusive side, missing
Hq[0] which cancels against Q in the tail).  Only same-bin pairs are
miscounted; each such pair's relu is < w, giving ~1e-6 final relative
error at NB=256 (validated against the exact reference in numpy).
count is exact (count = sum_{a<b} P_a Q_b from exact class totals).

This replaces the baseline's 64 full [128, 1024] pairwise compare tiles
(~8.4M compare elements + 48K PE columns per core) with 8 [128, 256]
own-row tiles + 2 [128, 1024] gather tiles (~0.8M elements, ~4K PE
columns) plus a tiny [8, 256] f32 AllReduce.

Sharding: each core owns a 1024-row slice of hazards/S/Y/c for both the
histogram build and the per-row gather/NLL; yc_full is replicated only
for the exact global P/Q class totals.  Final scalar is AllReduce-summed
on device, as before.
"""

import numpy as np

import concourse.mybir as mybir
import concourse.tile as tile
from concourse import bacc
from concourse.bass_utils import run_bass_kernel_spmd
from concourse.masks import make_identity

F32 = mybir.dt.float32
BF16 = mybir.dt.bfloat16
I32 = mybir.dt.int32
AF = mybir.ActivationFunctionType
ALU = mybir.AluOpType
AX = mybir.AxisListType

NCORES = 8
B, K = 8192, 4
SH = B // NCORES          # 1024 own rows (= own x-columns) per core
OWN = SH // 128           # 8 own 128-row chunks
NFULL = B // 128          # 64 chunks of the full batch (P/Q totals only)
NB = 256                  # risk bins
EBLK = NB // 128          # 2 edge partition-blocks
W = 4.0 / NB              # bin width: 2^-6, exact in bf16/f32
ALPHA = 0.15
RANKING_WEIGHT = 0.1
EPS = 1e-7

DO_COLLECTIVE = True


def _build_program():
    nc = bacc.Bacc(
        "TRN2",
        target_bir_lowering=False,
        debug=False,
        enable_asserts=False,
        num_devices=NCORES,
    )

    hs_own = nc.dram_tensor("hs_own", [2, SH, K], F32, kind="ExternalInput").ap()
    yc_own = nc.dram_tensor("yc_own", [2, SH], I32, kind="ExternalInput").ap()
    yc_full = nc.dram_tensor("yc_full", [2, B], I32, kind="ExternalInput").ap()
    out = nc.dram_tensor("out", [1, 1], F32, kind="ExternalOutput").ap()

    with tile.TileContext(nc) as tc:
        with (
            tc.tile_pool(name="const", bufs=1) as constp,
            tc.tile_pool(name="sb", bufs=1) as sb,
            tc.tile_pool(name="ps", bufs=1, space="PSUM") as ps,
            tc.tile_pool(name="pst", bufs=1, space="PSUM") as pst,
            tc.tile_pool(name="psrb", bufs=1, space="PSUM") as psrb,
            tc.tile_pool(name="dram", bufs=1, space="DRAM") as dramp,
        ):
            # ---------- input loads ----------
            # own-slice per-row tiles [p, t, jc, k], own row j = jc*128 + p
            hso = sb.tile([128, 2, OWN, K], F32)
            nc.sync.dma_start(hso[:], hs_own.rearrange("t (b p) k -> p t b k", p=128))
            hzo, so = hso[:, 0], hso[:, 1]
            yco = sb.tile([128, 2, OWN], I32)
            nc.sync.dma_start(yco[:], yc_own.rearrange("t (b p) -> p t b", p=128))
            yoi, coi = yco[:, 0, :], yco[:, 1, :]
            # full Y / c (for exact global P/Q class totals only)
            yc = sb.tile([128, 2, NFULL], I32)
            nc.sync.dma_start(yc[:], yc_full.rearrange("t (p b) -> p t b", p=128))
            yi, ci = yc[:, 0, :], yc[:, 1, :]

            # ---------- constants ----------
            # ACT function-table preload: emit a dummy Ln first so the table
            # load overlaps the input DMAs instead of the tail
            dumm = constp.tile([1, 1], F32)
            nc.vector.memset(dumm[:], 1.0)
            dumo = constp.tile([1, 1], F32)
            nc.scalar.activation(dumo[:], dumm[:], AF.Ln)

            ones1 = constp.tile([1, 128], F32)
            nc.vector.memset(ones1[:], 1.0)
            onescol = constp.tile([128, 1], F32)
            nc.vector.memset(onescol[:], 1.0)
            id8 = constp.tile([8, 8], F32)
            make_identity(nc, id8[:])
            id128 = constp.tile([128, 128], F32)
            make_identity(nc, id128[:])
            ones1b = constp.tile([1, 128], BF16)
            nc.vector.memset(ones1b[:], 1.0)
            # edge values along the free dim: E_{e+1} = (e+1)*w, bf16-exact
            erow_i = constp.tile([128, NB], I32)
            nc.gpsimd.iota(erow_i[:], [[1, NB]], base=1, channel_multiplier=0)
            erow = constp.tile([128, NB], BF16)
            nc.vector.tensor_scalar(erow[:], erow_i[:], W, None, op0=ALU.mult)
            # edge values along partitions: ecol[p, blk] = (blk*128 + p + 1)*w
            ecol_i = constp.tile([128, 1], I32)
            nc.gpsimd.iota(ecol_i[:], [[0, 1]], base=1, channel_multiplier=1)
            ecol = constp.tile([128, EBLK], F32)
            nc.vector.tensor_scalar(ecol[:, 0:1], ecol_i[:], W, None, op0=ALU.mult)
            nc.vector.tensor_scalar(
                ecol[:, 1:2], ecol[:, 0:1], 128.0 * W, None, op0=ALU.add
            )

            # shared PSUM scratch bank (disjoint views; 1 bank total)
            mA = pst.tile([128, 512], F32)

            # ---------- own-row scalars ----------
            ro = sb.tile([128, OWN], F32)
            nc.vector.tensor_reduce(ro[:], hzo, axis=AX.X, op=ALU.add)
            rt = sb.tile([128, OWN], BF16)  # canonical r~ = bf16(r)
            nc.vector.tensor_copy(rt[:], ro[:])
            rtf = sb.tile([128, OWN], F32)  # r~ upcast (f32 scalar operand)
            nc.vector.tensor_copy(rtf[:], rt[:])
            yof = sb.tile([128, OWN], F32)
            nc.vector.tensor_copy(yof[:], yoi)
            cobar = sb.tile([128, OWN], F32)  # 1 - c_own
            nc.vector.tensor_scalar(
                cobar[:], coi, -1.0, 1.0, op0=ALU.mult, op1=ALU.add
            )

            # U_own[p, jc, u]: u 0..3 = p^a = [Y=a][1-c], u 4..7 = q^a = [Y=a]
            Uo = sb.tile([128, OWN, 8], BF16)
            tmp_eq = sb.tile([128, OWN], F32)
            for a in range(4):
                nc.vector.tensor_scalar(
                    Uo[:, :, 4 + a], yof[:], float(a), None, op0=ALU.is_equal
                )
                nc.vector.tensor_scalar(
                    tmp_eq[:], yof[:], float(a), None, op0=ALU.is_equal
                )
                nc.vector.tensor_tensor(
                    Uo[:, :, a], tmp_eq[:], cobar[:], op=ALU.mult
                )

            # ---------- rb[p, x] = r~_x broadcast to all partitions ----------
            ps_rT = mA[0:8, 384:512]
            nc.tensor.transpose(ps_rT, ro[:], id128[:])
            row8 = sb.tile([8, 128], BF16)
            nc.vector.tensor_copy(row8[:], ps_rT)
            # repack [8, 128] -> [1, 1024] (x = jc*128 + q), then one
            # ones-outer-product matmul broadcasts r~ to all partitions
            row1 = sb.tile([1, SH], BF16)
            nc.sync.dma_start(row1[:], row8[:])
            ps_rb = psrb.tile([128, SH], F32)
            for ch in range(2):
                sl = slice(ch * 512, (ch + 1) * 512)
                nc.tensor.matmul(
                    ps_rb[:, sl], lhsT=ones1b[:], rhs=row1[0:1, sl],
                    start=True, stop=True,
                )
            rb = sb.tile([128, SH], BF16)
            nc.scalar.copy(rb[:], ps_rb[:])

            # ---------- phase B: own-row cumulative histogram ----------
            # CB[i, e] = [r~_i < (e+1)w]; F1[u, e] = sum_i U[i,u] CB[i,e]
            psF = ps.tile([8, NB], F32)
            for jc in range(OWN):
                cb = sb.tile([128, NB], BF16, tag=f"cb{jc}")
                nc.vector.tensor_scalar(
                    cb[:], erow[:], rtf[:, jc : jc + 1], None, op0=ALU.is_gt
                )
                nc.tensor.matmul(
                    psF[:],
                    lhsT=Uo[:, jc, :],
                    rhs=cb[:],
                    start=(jc == 0),
                    stop=(jc == OWN - 1),
                )
            F1s = sb.tile([8, NB], F32)
            nc.scalar.copy(F1s[:], psF[:])

            # AllReduce the partial histogram across the 8 cores
            ccF_in = dramp.tile([8, NB], F32)
            ccF_out = dramp.tile([8, NB], F32)
            nc.sync.dma_start(ccF_in[:], F1s[:])
            if DO_COLLECTIVE:
                nc.gpsimd.collective_compute(
                    "AllReduce",
                    ALU.add,
                    replica_groups=[list(range(NCORES))],
                    ins=[ccF_in.opt()],
                    outs=[ccF_out.opt()],
                )
                F1g_src = ccF_out
            else:
                F1g_src = ccF_in  # timing mode: same DMA path, no collective
            F1g = sb.tile([8, NB], F32)
            nc.sync.dma_start(F1g[:], F1g_src[:])

            # ---------- NLL (own rows; independent of the ranking term) ----
            gm = []
            for a in range(3):
                g = sb.tile([128, OWN], F32, tag=f"gm{a}")
                nc.vector.tensor_scalar(g[:], yof[:], float(a), None, op0=ALU.is_gt)
                gm.append(g)
            lm = {}
            for b in range(1, 4):
                l = sb.tile([128, OWN], F32, tag=f"lm{b}")
                nc.vector.tensor_scalar(l[:], yof[:], float(b), None, op0=ALU.is_lt)
                lm[b] = l

            e = []
            for k in range(4):
                ek = sb.tile([128, OWN], F32, tag=f"e{k}")
                nc.gpsimd.tensor_scalar(
                    ek[:], yof[:], float(k), None, op0=ALU.is_equal
                )
                e.append(ek)
            acc = sb.tile([128, OWN], F32)

            def gather(dst, src3, shift):
                # dst = sum_k e[k] * src3[:, :, k+shift] (skipping oob)
                first = True
                for k in range(4):
                    kk = k + shift
                    if kk < 0 or kk > 3:
                        continue
                    nc.gpsimd.tensor_tensor(
                        acc[:], e[k][:], src3[:, :, kk], op=ALU.mult
                    )
                    if first:
                        nc.gpsimd.tensor_copy(dst[:], acc[:])
                        first = False
                    else:
                        nc.gpsimd.tensor_tensor(dst[:], dst[:], acc[:], op=ALU.add)

            s_now = sb.tile([128, OWN], F32)
            gather(s_now, so, 0)
            h = sb.tile([128, OWN], F32)
            gather(h, hzo, 0)
            s_prev = sb.tile([128, OWN], F32)
            gather(s_prev, so, -1)  # e1*S0 + e2*S1 + e3*S2
            nc.gpsimd.tensor_tensor(s_prev[:], s_prev[:], e[0][:], op=ALU.add)

            for t in (s_now, h, s_prev):
                nc.gpsimd.tensor_scalar(t[:], t[:], EPS, None, op0=ALU.max)

            sph = sb.tile([128, OWN], F32)
            nc.gpsimd.tensor_tensor(sph[:], s_prev[:], h[:], op=ALU.mult)

            # ---------- exact global P/Q class totals (full Y/c) ----------
            yf = sb.tile([128, NFULL], F32)
            nc.vector.tensor_copy(yf[:], yi)
            cbar = sb.tile([128, NFULL], F32)  # 1 - c
            nc.vector.tensor_scalar(
                cbar[:], ci, -1.0, 1.0, op0=ALU.mult, op1=ALU.add
            )
            SS = sb.tile([128, 8], F32)
            eqf = sb.tile([128, NFULL], F32)
            pf = sb.tile([128, NFULL], F32)
            for a in range(4):
                nc.vector.tensor_scalar(
                    eqf[:], yf[:], float(a), None, op0=ALU.is_equal
                )
                nc.vector.tensor_reduce(
                    SS[:, 4 + a : 5 + a], eqf[:], axis=AX.X, op=ALU.add
                )
                nc.vector.tensor_tensor(pf[:], eqf[:], cbar[:], op=ALU.mult)
                nc.vector.tensor_reduce(
                    SS[:, a : a + 1], pf[:], axis=AX.X, op=ALU.add
                )
            ps_ss = mA[0:8, 0:1]
            nc.tensor.matmul(
                ps_ss, lhsT=SS[:], rhs=onescol[:], start=True, stop=True
            )
            ss_col = sb.tile([8, 1], F32)
            nc.vector.tensor_copy(ss_col[:], ps_ss)
            ps_row = mA[0:1, 8:16]
            nc.tensor.transpose(ps_row, ss_col[:], id8[:])
            pqk_row = sb.tile([1, 12], F32)  # cols 0:8 = P/Q, 8:12 = Hq[:,0]
            nc.vector.tensor_copy(pqk_row[:, 0:8], ps_row)

            # ---------- phase C compare tiles (gather by bin) ----------
            # CX[e, x] = [r~_x >= (e+1)w], e = blk*128 + p
            CX = sb.tile([128, EBLK, SH], BF16)
            for blk in range(EBLK):
                nc.vector.tensor_scalar(
                    CX[:, blk, :], rb[:], ecol[:, blk : blk + 1], None, op0=ALU.is_ge
                )

            # ---------- post-collective: H masses + shifted weights ----------
            # Hrow[u, t] = F1g[u, t] - F1g[u, t-1]; col NB kept 0 (pad)
            Hrow = sb.tile([8, NB + 1], F32)
            nc.vector.memset(Hrow[:, NB : NB + 1], 0.0)
            nc.vector.tensor_copy(Hrow[:, 0:1], F1g[:, 0:1])
            nc.vector.tensor_tensor(
                Hrow[:, 1:NB], F1g[:, 1:NB], F1g[:, 0 : NB - 1], op=ALU.subtract
            )
            # rhs_w[e, blk, 0:4] = Hp[blk*128+e]; [.., 4:8] = Hq[blk*128+e+1]
            # (PE inputs must start at partition 0: transpose all 8 u-rows for
            # both the unshifted and the e+1-shifted slice, keep half of each)
            def ps_w(blk, half):
                o = 64 + (blk * 2 + half) * 8
                return mA[:, o : o + 8]

            for blk in range(EBLK):
                nc.tensor.transpose(
                    ps_w(blk, 0),
                    Hrow[0:8, blk * 128 : (blk + 1) * 128],
                    id8[:],
                )
                nc.tensor.transpose(
                    ps_w(blk, 1),
                    Hrow[0:8, blk * 128 + 1 : (blk + 1) * 128 + 1],
                    id8[:],
                )
            rhs_w = sb.tile([128, EBLK, 8], BF16)
            for blk in range(EBLK):
                nc.vector.tensor_copy(rhs_w[:, blk, 0:4], ps_w(blk, 0)[:, 0:4])
                nc.vector.tensor_copy(rhs_w[:, blk, 4:8], ps_w(blk, 1)[:, 4:8])
            # Hq[:, 0] (cancels against Q in the tail) -> broadcast row
            ps_h0 = mA[0:1, 16:24]
            nc.tensor.transpose(ps_h0, Hrow[:, 0:1], id8[:])
            nc.vector.tensor_copy(pqk_row[:, 8:12], ps_h0[0:1, 4:8])

            ps_bc = mA[:, 32:44]
            nc.tensor.matmul(
                ps_bc, lhsT=ones1[:], rhs=pqk_row[:], start=True, stop=True
            )
            QBK = sb.tile([128, 12], F32)  # [:,0:4]=P, [:,4:8]=Q, [:,8:12]=Hq0
            nc.vector.tensor_copy(QBK[:], ps_bc)
            QmH0 = sb.tile([128, 4], F32)  # Q_b - Hq[0, b]
            nc.vector.tensor_tensor(
                QmH0[:], QBK[:, 4:8], QBK[:, 8:12], op=ALU.subtract
            )

            # count = sum_{a<b} P_a Q_b -> rscale = 0.1/count (all partitions)
            sfx = sb.tile([128, 3], F32)
            nc.gpsimd.tensor_copy(sfx[:, 2:3], QBK[:, 7:8])
            nc.gpsimd.tensor_tensor(sfx[:, 1:2], QBK[:, 6:7], QBK[:, 7:8], op=ALU.add)
            nc.gpsimd.tensor_tensor(sfx[:, 0:1], QBK[:, 5:6], sfx[:, 1:2], op=ALU.add)
            cnt = sb.tile([128, 3], F32)
            nc.gpsimd.tensor_tensor(cnt[:], QBK[:, 0:3], sfx[:], op=ALU.mult)
            cnt1 = sb.tile([128, 1], F32)
            nc.vector.tensor_reduce(cnt1[:], cnt[:], axis=AX.X, op=ALU.add)
            rscale = sb.tile([128, 1], F32)
            nc.vector.reciprocal(rscale[:], cnt1[:])
            nc.vector.tensor_scalar(
                rscale[:], rscale[:], RANKING_WEIGHT, None, op0=ALU.mult
            )

            # ---------- V gather: V[x, jc, u] = sum_e CX[e, x] rhs_w[e, u] ----
            psV = ps.tile([128, OWN, 8], F32, tag="V")
            for jc in range(OWN):
                for blk in range(EBLK):
                    nc.tensor.matmul(
                        psV[:, jc, :],
                        lhsT=CX[:, blk, jc * 128 : (jc + 1) * 128],
                        rhs=rhs_w[:, blk, :],
                        start=(blk == 0),
                        stop=(blk == EBLK - 1),
                    )
            Vt = sb.tile([128, OWN, 8], F32)
            nc.vector.tensor_copy(Vt[:], psV[:])

            # ---------- NLL logs ----------
            lnsh = sb.tile([128, OWN], F32)
            nc.scalar.activation(lnsh[:], sph[:], AF.Ln)
            lnsn = sb.tile([128, OWN], F32)
            nc.scalar.activation(lnsn[:], s_now[:], AF.Ln)
            # L = -cbar*lnsh - 0.85*lnsn + 0.85*cbar*lnsn
            Lt = sb.tile([128, OWN], F32)
            nc.gpsimd.tensor_tensor(Lt[:], cobar[:], lnsh[:], op=ALU.mult)
            t3 = sb.tile([128, OWN], F32)
            nc.gpsimd.tensor_tensor(t3[:], cobar[:], lnsn[:], op=ALU.mult)
            nc.gpsimd.tensor_scalar(
                t3[:], t3[:], 1.0 - ALPHA, None, op0=ALU.mult
            )
            nc.gpsimd.tensor_tensor(Lt[:], t3[:], Lt[:], op=ALU.subtract)
            nc.gpsimd.tensor_scalar(
                t3[:], lnsn[:], 1.0 - ALPHA, None, op0=ALU.mult
            )
            nc.gpsimd.tensor_tensor(Lt[:], Lt[:], t3[:], op=ALU.subtract)

            # ---------- tail: T1/T2 from V ----------
            # T1 = sum_{a<Y_x} V_A[a];  T2' = sum_{b>Y_x} (V_<='[b] - (Q-Hq0)_b)
            t1p = []
            for a in range(3):
                t = sb.tile([128, OWN], F32, tag=f"t1p{a}")
                nc.vector.tensor_tensor(t[:], Vt[:, :, a], gm[a][:], op=ALU.mult)
                t1p.append(t)
            t2p = []
            for b in range(1, 4):
                t = sb.tile([128, OWN], F32, tag=f"t2p{b}")
                nc.vector.scalar_tensor_tensor(
                    t[:], Vt[:, :, 4 + b], QmH0[:, b : b + 1], lm[b][:],
                    op0=ALU.subtract, op1=ALU.mult,
                )
                t2p.append(t)
            T1 = sb.tile([128, OWN], F32)
            T2 = sb.tile([128, OWN], F32)
            nc.vector.tensor_tensor(T1[:], t1p[0][:], t1p[1][:], op=ALU.add)
            nc.vector.tensor_tensor(T1[:], T1[:], t1p[2][:], op=ALU.add)
            nc.gpsimd.tensor_tensor(T2[:], t2p[0][:], t2p[1][:], op=ALU.add)
            nc.gpsimd.tensor_tensor(T2[:], T2[:], t2p[2][:], op=ALU.add)

            # contrib = r * (T1 + cbar * T2')   (T2' = -T2_true)
            contrib = sb.tile([128, OWN], F32)
            nc.vector.tensor_tensor(contrib[:], cobar[:], T2[:], op=ALU.mult)
            nc.vector.tensor_tensor(contrib[:], T1[:], contrib[:], op=ALU.add)
            nc.vector.tensor_tensor(contrib[:], contrib[:], ro[:], op=ALU.mult)

            # grand = L/B + contrib * (0.1/count); reduce to a single scalar
            grand = sb.tile([128, OWN], F32)
            nc.vector.tensor_scalar(
                contrib[:], contrib[:], rscale[:, 0:1], None, op0=ALU.mult
            )
            red = sb.tile([128, 1], F32)
            nc.vector.scalar_tensor_tensor(
                grand[:], Lt[:], 1.0 / B, contrib[:],
                op0=ALU.mult, op1=ALU.add, accum_out=red[:],
            )
            ps_fin = mA[0:1, 100:101]
            nc.tensor.matmul(
                ps_fin, lhsT=red[:], rhs=onescol[:], start=True, stop=True
            )
            partial = sb.tile([1, 1], F32)
            nc.vector.tensor_copy(partial[:], ps_fin)

            # ---------- global sum ----------
            cc_in = dramp.tile([1, 1], F32)
            cc_out = dramp.tile([1, 1], F32)
            nc.sync.dma_start(cc_in[:], partial[:])
            if DO_COLLECTIVE:
                nc.gpsimd.collective_compute(
                    "AllReduce",
                    ALU.add,
                    replica_groups=[list(range(NCORES))],
                    ins=[cc_in.opt()],
                    outs=[cc_out.opt()],
                )
                nc.sync.dma_start(out[:], cc_out[:])
            else:
                nc.sync.dma_start(out[:], cc_in[:])

    nc.compile()
    return nc


_PROGRAM = None


def _get_program():
    global _PROGRAM
    if _PROGRAM is None:
        _PROGRAM = _build_program()
    return _PROGRAM


def kernel(hazards, S, Y, c):
    hazards = np.ascontiguousarray(np.asarray(hazards, dtype=np.float32))
    S = np.ascontiguousarray(np.asarray(S, dtype=np.float32))
    Y32 = np.asarray(Y).astype(np.int32)
    c32 = np.asarray(c).astype(np.int32)
    yc_full = np.ascontiguousarray(np.stack([Y32, c32]))

    nc = _get_program()
    in_maps = []
    for m in range(NCORES):
        sl = slice(m * SH, (m + 1) * SH)
        in_maps.append(
            {
                "hs_own": np.ascontiguousarray(
                    np.stack([hazards[sl], S[sl]])
                ),
                "yc_own": np.ascontiguousarray(yc_full[:, sl]),
                "yc_full": yc_full,
            }
        )
    res = run_bass_kernel_spmd(nc, in_maps, core_ids=list(range(NCORES)))
    if DO_COLLECTIVE:
        val = res.results[0]["out"][0, 0]
    else:
        val = np.float32(sum(r["out"][0, 0] for r in res.results))
    return np.asarray(val, dtype=np.float32).reshape(())


# revision 14
# speedup vs baseline: 1.5298x; 1.0463x over previous
"""CombinedSurvLoss (NLL survival + pairwise ranking) on 8 TRN2 NeuronCores.

Math
----
reference = mean_j L_j + 0.1 * total / count, where

  L_j     = -(1-c_j) * ln(clip(s_prev_j) * clip(h_j)) - 0.85 * c_j * ln(clip(s_now_j))
  total   = sum_{i,j} [c_i=0][Y_j>Y_i] relu(r_j - r_i),  r = hazards.sum(axis=1)
  count   = sum_{i,j} [c_i=0][Y_j>Y_i]

Binned-rank decomposition of the O(B^2) term
--------------------------------------------
Quantize the (bf16-rounded, canonical) risk r~ into NB uniform bins of
width w over [0, 4).  With per-class indicator weights
p^a_i = [Y_i=a][c_i=0], q^b_i = [Y_i=b] build the per-bin mass table

  H[u, t] = sum_i u_i [t*w <= r~_i < (t+1)*w]        (u over the 8 classes)

via one compare tile per own 128-row chunk (CB[i, e] = [r~_i < (e+1)w],
contracted with U_own on the TensorEngine -> cumulative F1[u, e],
AllReduced across the 8 cores as a tiny [8, NB] f32 table, then
differenced).  A pair (i, j) is counted iff bin_i < bin_j, consistently
on both sides of the decomposition

  total ~= sum_x r_x * ( sum_{a<Y_x} V_A[p^a, x]
                         - [c_x=0] sum_{b>Y_x} (Q_b - V_<=[q^b, x]) )

where V_A[u, x] = sum_t H[u,t][t <  bin_x]  (strict prefix)
      V_<=[u,x] = sum_t H[u,t][t <= bin_x]  (inclusive prefix)

Both gathers share one compare tile set CX[e, x] = [r~_x >= e*w]
(e on partitions; row e=0 is identically 1, absorbing the inclusive
prefix's H[0] term) and ONE PE contraction with shifted weight columns:
cols 0:4 = Hp[e-1] (strict side), cols 4:8 = Hq[e] (inclusive side).
Only same-bin pairs are miscounted; each such pair's relu is < w,
giving ~1e-6 final relative error at NB=256 (validated against the
exact reference in numpy).  count stays exact (sum_{a<b} P_a Q_b from
exact class totals), as does the Q-side suffix gather QLM.

This replaces the baseline's 64 full [128, 1024] pairwise compare tiles
(~8.4M compare elements + 48K PE columns per core) with 8 [128, 256]
own-row tiles + 2 [128, 1024] gather tiles (~0.8M elements, ~4K PE
columns) plus the [8, NB] f32 AllReduce.

Sharding: each core owns a 1024-row slice of hazards/S/Y/c for both the
histogram build and the per-row gather/NLL; yc_full is replicated only
for the exact global P/Q class totals.  Final scalar is AllReduce-summed
on device, as before.
"""

import numpy as np

import concourse.mybir as mybir
import concourse.tile as tile
from concourse import bacc
from concourse.bass_utils import run_bass_kernel_spmd
from concourse.masks import make_identity

F32 = mybir.dt.float32
BF16 = mybir.dt.bfloat16
I32 = mybir.dt.int32
AF = mybir.ActivationFunctionType
ALU = mybir.AluOpType
AX = mybir.AxisListType

NCORES = 8
B, K = 8192, 4
SH = B // NCORES          # 1024 own rows (= own x-columns) per core
OWN = SH // 128           # 8 own 128-row chunks
NFULL = B // 128          # 64 chunks of the full batch (P/Q totals only)
NB = 256                  # risk bins
EBLK = NB // 128          # 2 edge partition-blocks
W = 4.0 / NB              # bin width: 2^-6, exact in bf16/f32
ALPHA = 0.15
RANKING_WEIGHT = 0.1
EPS = 1e-7

DO_COLLECTIVE = True


def _build_program():
    nc = bacc.Bacc(
        "TRN2",
        target_bir_lowering=False,
        debug=False,
        enable_asserts=False,
        num_devices=NCORES,
    )

    hz_own = nc.dram_tensor("hz_own", [SH, K], F32, kind="ExternalInput").ap()
    s_own = nc.dram_tensor("s_own", [SH, K], F32, kind="ExternalInput").ap()
    yc_own = nc.dram_tensor("yc_own", [2, SH], I32, kind="ExternalInput").ap()
    yc_full = nc.dram_tensor("yc_full", [2, B], I32, kind="ExternalInput").ap()
    out = nc.dram_tensor("out", [1, 1], F32, kind="ExternalOutput").ap()

    with tile.TileContext(nc) as tc:
        with (
            tc.tile_pool(name="const", bufs=1) as constp,
            tc.tile_pool(name="sb", bufs=1) as sb,
            tc.tile_pool(name="ps", bufs=1, space="PSUM") as ps,
            tc.tile_pool(name="pst", bufs=1, space="PSUM") as pst,
            tc.tile_pool(name="psrb", bufs=1, space="PSUM") as psrb,
            tc.tile_pool(name="dram", bufs=1, space="DRAM") as dramp,
        ):
            # ---------- input loads (criticality order; transfers serialize
            # on the DMA engines, so the risk chain's hazards go first) ------
            hzo = sb.tile([128, OWN, K], F32)
            nc.sync.dma_start(hzo[:], hz_own.rearrange("(b p) k -> p b k", p=128))
            yco = sb.tile([128, 2, OWN], I32)
            nc.sync.dma_start(yco[:], yc_own.rearrange("t (b p) -> p t b", p=128))
            yoi, coi = yco[:, 0, :], yco[:, 1, :]
            so = sb.tile([128, OWN, K], F32)
            nc.sync.dma_start(so[:], s_own.rearrange("(b p) k -> p b k", p=128))
            yc = sb.tile([128, 2, NFULL], I32)
            nc.sync.dma_start(yc[:], yc_full.rearrange("t (p b) -> p t b", p=128))
            yi, ci = yc[:, 0, :], yc[:, 1, :]

            # ---------- constants (fill the DMA-latency window) ----------
            # ACT function-table preload: dummy Ln so the load overlaps DMAs
            dumm = constp.tile([1, 1], F32)
            nc.vector.memset(dumm[:], 1.0)
            dumo = constp.tile([1, 1], F32)
            nc.scalar.activation(dumo[:], dumm[:], AF.Ln)

            ones1b = constp.tile([1, 128], BF16)
            nc.vector.memset(ones1b[:], 1.0)
            ones1 = constp.tile([1, 128], F32)
            nc.vector.memset(ones1[:], 1.0)
            onescol = constp.tile([128, 1], F32)
            nc.vector.memset(onescol[:], 1.0)
            id8 = constp.tile([8, 8], F32)
            make_identity(nc, id8[:])
            id128 = constp.tile([128, 128], F32)
            make_identity(nc, id128[:])
            # CB edges along free dim: (e+1)*w, bf16-exact
            erow_i = constp.tile([128, NB], I32)
            nc.gpsimd.iota(erow_i[:], [[1, NB]], base=1, channel_multiplier=0)
            erow = constp.tile([128, NB], BF16)
            nc.vector.tensor_scalar(erow[:], erow_i[:], W, None, op0=ALU.mult)
            # CX edges along partitions: ecol[p, blk] = (blk*128 + p)*w
            # (row 0 of block 0 is edge 0 -> an all-ones CX row)
            ecol_i = constp.tile([128, 1], I32)
            nc.gpsimd.iota(ecol_i[:], [[0, 1]], base=0, channel_multiplier=1)
            ecol = constp.tile([128, EBLK], F32)
            nc.vector.tensor_scalar(ecol[:, 0:1], ecol_i[:], W, None, op0=ALU.mult)
            nc.vector.tensor_scalar(
                ecol[:, 1:2], ecol[:, 0:1], 128.0 * W, None, op0=ALU.add
            )
            # H pad tile: col 1+t = H[t], col 0 stays 0 (memset now, no deps)
            Hpad = sb.tile([8, NB + 1], F32)
            nc.vector.memset(Hpad[:, 0:1], 0.0)

            # shared PSUM scratch bank (disjoint views; 1 bank total)
            mA = pst.tile([128, 512], F32)

            # ---------- risk chain (critical path to the histogram) ----------
            ro = sb.tile([128, OWN], F32)
            nc.vector.tensor_reduce(ro[:], hzo[:], axis=AX.X, op=ALU.add)
            rt = sb.tile([128, OWN], BF16)  # canonical r~ = bf16(r)
            nc.vector.tensor_copy(rt[:], ro[:])
            rtf = sb.tile([128, OWN], F32)  # r~ upcast (f32 scalar operand)
            nc.vector.tensor_copy(rtf[:], rt[:])

            # rb broadcast chain (needed only by CX, well before the gather):
            # transpose r -> [8, 128], round to bf16, repack to one [1, 1024]
            # row by DMA, broadcast to all partitions via ones outer product
            ps_rT = mA[0:8, 384:512]
            nc.tensor.transpose(ps_rT, ro[:], id128[:])
            row8 = sb.tile([8, 128], BF16)
            nc.vector.tensor_copy(row8[:], ps_rT)
            row1 = sb.tile([1, SH], BF16)
            nc.sync.dma_start(row1[:], row8[:])

            yof = sb.tile([128, OWN], F32)
            nc.vector.tensor_copy(yof[:], yoi)
            cobar = sb.tile([128, OWN], F32)  # 1 - c_own
            nc.vector.tensor_scalar(
                cobar[:], coi, -1.0, 1.0, op0=ALU.mult, op1=ALU.add
            )

            # U_own[p, jc, u]: u 0..3 = p^a = [Y=a][1-c], u 4..7 = q^a = [Y=a]
            Uo = sb.tile([128, OWN, 8], BF16)
            tmp_eq = sb.tile([128, OWN], F32)
            for a in range(4):
                nc.vector.tensor_scalar(
                    Uo[:, :, 4 + a], yof[:], float(a), None, op0=ALU.is_equal
                )
                nc.vector.tensor_scalar(
                    tmp_eq[:], yof[:], float(a), None, op0=ALU.is_equal
                )
                nc.vector.tensor_tensor(
                    Uo[:, :, a], tmp_eq[:], cobar[:], op=ALU.mult
                )

            # ---------- phase B: own-row cumulative histogram ----------
            # CB[i, e] = [r~_i < (e+1)w]; F1[u, e] = sum_i U[i,u] CB[i,e]
            psF = ps.tile([8, NB], F32)
            for jc in range(OWN):
                cb = sb.tile([128, NB], BF16, tag=f"cb{jc}")
                nc.vector.tensor_scalar(
                    cb[:], erow[:], rtf[:, jc : jc + 1], None, op0=ALU.is_gt
                )
                nc.tensor.matmul(
                    psF[:],
                    lhsT=Uo[:, jc, :],
                    rhs=cb[:],
                    start=(jc == 0),
                    stop=(jc == OWN - 1),
                )

            # AllReduce the partial histogram across the 8 cores
            F1s = sb.tile([8, NB], F32)
            nc.scalar.copy(F1s[:], psF[:])
            ccF_in = dramp.tile([8, NB], F32)
            ccF_out = dramp.tile([8, NB], F32)
            nc.sync.dma_start(ccF_in[:], F1s[:])
            if DO_COLLECTIVE:
                nc.gpsimd.collective_compute(
                    "AllReduce",
                    ALU.add,
                    replica_groups=[list(range(NCORES))],
                    ins=[ccF_in.opt()],
                    outs=[ccF_out.opt()],
                )
                F1g_src = ccF_out
            else:
                F1g_src = ccF_in  # timing mode: same DMA path, no collective
            F1g = sb.tile([8, NB], F32)
            nc.sync.dma_start(F1g[:], F1g_src[:])

            # rb materialization + gather compare tiles (pre-collective)
            ps_rb = psrb.tile([128, SH], F32)
            for ch in range(2):
                sl = slice(ch * 512, (ch + 1) * 512)
                nc.tensor.matmul(
                    ps_rb[:, sl], lhsT=ones1b[:], rhs=row1[0:1, sl],
                    start=True, stop=True,
                )
            rb = sb.tile([128, SH], BF16)
            nc.scalar.copy(rb[:], ps_rb[:])
            # CX[e, x] = [r~_x >= e*w]
            CX = sb.tile([128, EBLK, SH], BF16)
            for blk in range(EBLK):
                nc.vector.tensor_scalar(
                    CX[:, blk, :], rb[:], ecol[:, blk : blk + 1], None, op0=ALU.is_ge
                )

            # Y-comparison mask stacks for the fused tail reduction
            GM = sb.tile([128, OWN, 3], F32)   # [Y > a], a = 0..2
            for a in range(3):
                nc.vector.tensor_scalar(
                    GM[:, :, a], yof[:], float(a), None, op0=ALU.is_gt
                )
            LM = sb.tile([128, OWN, 3], F32)   # [Y < b], b = 1..3
            for b in range(1, 4):
                nc.vector.tensor_scalar(
                    LM[:, :, b - 1], yof[:], float(b), None, op0=ALU.is_lt
                )

            # ---------- NLL (gpsimd; overlaps everything above) ----------
            e = []
            for k in range(4):
                ek = sb.tile([128, OWN], F32, tag=f"e{k}")
                nc.gpsimd.tensor_scalar(
                    ek[:], yof[:], float(k), None, op0=ALU.is_equal
                )
                e.append(ek)
            acc = sb.tile([128, OWN], F32)

            def gather(dst, src3, shift):
                # dst = sum_k e[k] * src3[:, :, k+shift] (skipping oob)
                first = True
                for k in range(4):
                    kk = k + shift
                    if kk < 0 or kk > 3:
                        continue
                    nc.gpsimd.tensor_tensor(
                        acc[:], e[k][:], src3[:, :, kk], op=ALU.mult
                    )
                    if first:
                        nc.gpsimd.tensor_copy(dst[:], acc[:])
                        first = False
                    else:
                        nc.gpsimd.tensor_tensor(dst[:], dst[:], acc[:], op=ALU.add)

            s_now = sb.tile([128, OWN], F32)
            gather(s_now, so, 0)
            h = sb.tile([128, OWN], F32)
            gather(h, hzo, 0)
            s_prev = sb.tile([128, OWN], F32)
            gather(s_prev, so, -1)  # e1*S0 + e2*S1 + e3*S2
            nc.gpsimd.tensor_tensor(s_prev[:], s_prev[:], e[0][:], op=ALU.add)

            for t in (s_now, h, s_prev):
                nc.gpsimd.tensor_scalar(t[:], t[:], EPS, None, op0=ALU.max)

            sph = sb.tile([128, OWN], F32)
            nc.gpsimd.tensor_tensor(sph[:], s_prev[:], h[:], op=ALU.mult)

            # ---------- exact global P/Q class totals (full Y/c, gpsimd) ----
            yf = sb.tile([128, NFULL], F32)
            nc.gpsimd.tensor_copy(yf[:], yi)
            cbar = sb.tile([128, NFULL], F32)  # 1 - c
            nc.gpsimd.tensor_scalar(
                cbar[:], ci, -1.0, 1.0, op0=ALU.mult, op1=ALU.add
            )
            SS = sb.tile([128, 8], F32)
            eqf = sb.tile([128, NFULL], F32)
            pf = sb.tile([128, NFULL], F32)
            for a in range(4):
                nc.gpsimd.tensor_scalar(
                    eqf[:], yf[:], float(a), None, op0=ALU.is_equal
                )
                nc.vector.tensor_reduce(
                    SS[:, 4 + a : 5 + a], eqf[:], axis=AX.X, op=ALU.add
                )
                nc.gpsimd.tensor_tensor(pf[:], eqf[:], cbar[:], op=ALU.mult)
                nc.vector.tensor_reduce(
                    SS[:, a : a + 1], pf[:], axis=AX.X, op=ALU.add
                )
            ps_ss = mA[0:8, 0:1]
            nc.tensor.matmul(
                ps_ss, lhsT=SS[:], rhs=onescol[:], start=True, stop=True
            )
            ss_col = sb.tile([8, 1], F32)
            nc.vector.tensor_copy(ss_col[:], ps_ss)
            ps_row = mA[0:1, 8:16]
            nc.tensor.transpose(ps_row, ss_col[:], id8[:])
            pqk_row = sb.tile([1, 8], F32)  # P_0..3, Q_0..3
            nc.vector.tensor_copy(pqk_row[:], ps_row)
            ps_bc = mA[:, 32:40]
            nc.tensor.matmul(
                ps_bc, lhsT=ones1[:], rhs=pqk_row[:], start=True, stop=True
            )
            QBK = sb.tile([128, 8], F32)  # [:,0:4]=P, [:,4:8]=Q on all parts
            nc.vector.tensor_copy(QBK[:], ps_bc)

            # count = sum_{a<b} P_a Q_b; sfx[k] = sum_{b>k} Q_b
            sfx = sb.tile([128, 3], F32)
            nc.gpsimd.tensor_copy(sfx[:, 2:3], QBK[:, 7:8])
            nc.gpsimd.tensor_tensor(sfx[:, 1:2], QBK[:, 6:7], QBK[:, 7:8], op=ALU.add)
            nc.gpsimd.tensor_tensor(sfx[:, 0:1], QBK[:, 5:6], sfx[:, 1:2], op=ALU.add)
            cnt = sb.tile([128, 3], F32)
            nc.gpsimd.tensor_tensor(cnt[:], QBK[:, 0:3], sfx[:], op=ALU.mult)
            cnt1 = sb.tile([128, 1], F32)
            nc.vector.tensor_reduce(cnt1[:], cnt[:], axis=AX.X, op=ALU.add)
            rscale = sb.tile([128, 1], F32)
            nc.vector.reciprocal(rscale[:], cnt1[:])
            nc.vector.tensor_scalar(
                rscale[:], rscale[:], RANKING_WEIGHT, None, op0=ALU.mult
            )
            # QLM = sum_{b > Y_x} Q_b (exact, from the NLL one-hots)
            qa = sb.tile([128, OWN], F32)
            nc.vector.tensor_scalar(
                qa[:], e[0][:], sfx[:, 0:1], None, op0=ALU.mult
            )
            qb = sb.tile([128, OWN], F32)
            nc.vector.scalar_tensor_tensor(
                qb[:], e[1][:], sfx[:, 1:2], qa[:], op0=ALU.mult, op1=ALU.add
            )
            QLM = sb.tile([128, OWN], F32)
            nc.vector.scalar_tensor_tensor(
                QLM[:], e[2][:], sfx[:, 2:3], qb[:], op0=ALU.mult, op1=ALU.add
            )

            # ---------- NLL logs (ACT) + L assembly (gpsimd) ----------
            lnsh = sb.tile([128, OWN], F32)
            nc.scalar.activation(lnsh[:], sph[:], AF.Ln)
            lnsn = sb.tile([128, OWN], F32)
            nc.scalar.activation(lnsn[:], s_now[:], AF.Ln)
            # L = -cbar*lnsh - 0.85*lnsn + 0.85*cbar*lnsn
            Lt = sb.tile([128, OWN], F32)
            nc.gpsimd.tensor_tensor(Lt[:], cobar[:], lnsh[:], op=ALU.mult)
            t3 = sb.tile([128, OWN], F32)
            nc.gpsimd.tensor_tensor(t3[:], cobar[:], lnsn[:], op=ALU.mult)
            nc.gpsimd.tensor_scalar(
                t3[:], t3[:], 1.0 - ALPHA, None, op0=ALU.mult
            )
            nc.gpsimd.tensor_tensor(Lt[:], t3[:], Lt[:], op=ALU.subtract)
            nc.gpsimd.tensor_scalar(
                t3[:], lnsn[:], 1.0 - ALPHA, None, op0=ALU.mult
            )
            nc.gpsimd.tensor_tensor(Lt[:], Lt[:], t3[:], op=ALU.subtract)

            # ---------- post-collective: H masses + shifted weights ----------
            nc.vector.tensor_copy(Hpad[:, 1:2], F1g[:, 0:1])
            nc.vector.tensor_tensor(
                Hpad[:, 2 : NB + 1], F1g[:, 1:NB], F1g[:, 0 : NB - 1],
                op=ALU.subtract,
            )

            def ps_w(blk, half):
                o = 64 + (blk * 2 + half) * 8
                return mA[:, o : o + 8]

            for blk in range(EBLK):
                # strict side: rhs_A[e] = H[e-1]  (Hpad col offset 0)
                nc.tensor.transpose(
                    ps_w(blk, 0),
                    Hpad[0:8, blk * 128 : blk * 128 + 128],
                    id8[:],
                )
                # inclusive side: rhs_B[e] = H[e]  (Hpad col offset 1)
                nc.tensor.transpose(
                    ps_w(blk, 1),
                    Hpad[0:8, blk * 128 + 1 : blk * 128 + 129],
                    id8[:],
                )
            rhs_w = sb.tile([128, EBLK, 8], BF16)
            for blk in range(EBLK):
                nc.vector.tensor_copy(rhs_w[:, blk, 0:4], ps_w(blk, 0)[:, 0:4])
                nc.vector.tensor_copy(rhs_w[:, blk, 4:8], ps_w(blk, 1)[:, 4:8])

            # ---------- V gather: V[x, jc, u] = sum_e CX[e, x] rhs_w[e, u] ----
            psV = ps.tile([128, OWN, 8], F32, tag="V")
            for jc in range(OWN):
                for blk in range(EBLK):
                    nc.tensor.matmul(
                        psV[:, jc, :],
                        lhsT=CX[:, blk, jc * 128 : (jc + 1) * 128],
                        rhs=rhs_w[:, blk, :],
                        start=(blk == 0),
                        stop=(blk == EBLK - 1),
                    )
            Vt = sb.tile([128, OWN, 8], F32)
            nc.vector.tensor_copy(Vt[:], psV[:])

            # ---------- fused tail (single engine, last-dim reductions) ----
            # T1 = sum_a GM[.,a] * V_A[a];  T2' = sum_b LM[.,b] * V<=[b] - QLM
            TM = sb.tile([128, OWN, 3], F32)
            nc.vector.tensor_tensor(TM[:], GM[:], Vt[:, :, 0:3], op=ALU.mult)
            T1 = sb.tile([128, OWN], F32)
            nc.vector.tensor_reduce(T1[:], TM[:], axis=AX.X, op=ALU.add)
            nc.vector.tensor_tensor(TM[:], LM[:], Vt[:, :, 5:8], op=ALU.mult)
            T2 = sb.tile([128, OWN], F32)
            nc.vector.tensor_reduce(T2[:], TM[:], axis=AX.X, op=ALU.add)
            nc.vector.tensor_tensor(T2[:], T2[:], QLM[:], op=ALU.subtract)

            # contrib = r * (T1 + cbar * T2') * (0.1/count)
            contrib = sb.tile([128, OWN], F32)
            nc.vector.tensor_tensor(contrib[:], cobar[:], T2[:], op=ALU.mult)
            nc.vector.tensor_tensor(contrib[:], T1[:], contrib[:], op=ALU.add)
            nc.vector.tensor_tensor(contrib[:], contrib[:], ro[:], op=ALU.mult)
            nc.vector.tensor_scalar(
                contrib[:], contrib[:], rscale[:, 0:1], None, op0=ALU.mult
            )
            grand = sb.tile([128, OWN], F32)
            red = sb.tile([128, 1], F32)
            nc.vector.scalar_tensor_tensor(
                grand[:], Lt[:], 1.0 / B, contrib[:],
                op0=ALU.mult, op1=ALU.add, accum_out=red[:],
            )
            ps_fin = mA[0:1, 100:101]
            nc.tensor.matmul(
                ps_fin, lhsT=red[:], rhs=onescol[:], start=True, stop=True
            )

            # ---------- global sum ----------
            partial = sb.tile([1, 1], F32)
            nc.vector.tensor_copy(partial[:], ps_fin)
            cc_in = dramp.tile([1, 1], F32)
            cc_out = dramp.tile([1, 1], F32)
            nc.sync.dma_start(cc_in[:], partial[:])
            if DO_COLLECTIVE:
                nc.gpsimd.collective_compute(
                    "AllReduce",
                    ALU.add,
                    replica_groups=[list(range(NCORES))],
                    ins=[cc_in.opt()],
                    outs=[cc_out.opt()],
                )
                nc.sync.dma_start(out[:], cc_out[:])
            else:
                nc.sync.dma_start(out[:], cc_in[:])

    nc.compile()
    return nc


_PROGRAM = None


def _get_program():
    global _PROGRAM
    if _PROGRAM is None:
        _PROGRAM = _build_program()
    return _PROGRAM


def kernel(hazards, S, Y, c):
    hazards = np.ascontiguousarray(np.asarray(hazards, dtype=np.float32))
    S = np.ascontiguousarray(np.asarray(S, dtype=np.float32))
    Y32 = np.asarray(Y).astype(np.int32)
    c32 = np.asarray(c).astype(np.int32)
    yc_full = np.ascontiguousarray(np.stack([Y32, c32]))

    nc = _get_program()
    in_maps = []
    for m in range(NCORES):
        sl = slice(m * SH, (m + 1) * SH)
        in_maps.append(
            {
                "hz_own": np.ascontiguousarray(hazards[sl]),
                "s_own": np.ascontiguousarray(S[sl]),
                "yc_own": np.ascontiguousarray(yc_full[:, sl]),
                "yc_full": yc_full,
            }
        )
    res = run_bass_kernel_spmd(nc, in_maps, core_ids=list(range(NCORES)))
    if DO_COLLECTIVE:
        val = res.results[0]["out"][0, 0]
    else:
        val = np.float32(sum(r["out"][0, 0] for r in res.results))
    return np.asarray(val, dtype=np.float32).reshape(())


# revision 15
# speedup vs baseline: 1.7188x; 1.1235x over previous
"""CombinedSurvLoss (NLL survival + pairwise ranking) on 8 TRN2 NeuronCores.

Math
----
reference = mean_j L_j + 0.1 * total / count, where

  L_j     = -(1-c_j) * ln(clip(s_prev_j) * clip(h_j)) - 0.85 * c_j * ln(clip(s_now_j))
  total   = sum_{i,j} [c_i=0][Y_j>Y_i] relu(r_j - r_i),  r = hazards.sum(axis=1)
  count   = sum_{i,j} [c_i=0][Y_j>Y_i]

Binned-rank decomposition of the O(B^2) term
--------------------------------------------
Quantize the (bf16-rounded, canonical) risk r~ into NB uniform bins of
width w over [0, 4).  With per-class indicator weights
p^a_i = [Y_i=a][c_i=0], q^b_i = [Y_i=b] build the per-bin mass table

  H[u, t] = sum_i u_i [t*w <= r~_i < (t+1)*w]        (u over the 8 classes)

via one compare tile per own 128-row chunk (CB[i, e] = [r~_i < (e+1)w],
contracted with U_own on the TensorEngine -> cumulative F1[u, e],
AllReduced across the 8 cores as a tiny [8, NB] f32 table, then
differenced).  A pair (i, j) is counted iff bin_i < bin_j, consistently
on both sides of the decomposition

  total ~= sum_x r_x * ( sum_{a<Y_x} V_A[p^a, x]
                         - [c_x=0] sum_{b>Y_x} (Q_b - V_<=[q^b, x]) )

where V_A[u, x] = sum_t H[u,t][t <  bin_x]  (strict prefix)
      V_<=[u,x] = sum_t H[u,t][t <= bin_x]  (inclusive prefix)

Both gathers share one compare tile set CX[e, x] = [r~_x >= e*w]
(e on partitions; row e=0 is identically 1, absorbing the inclusive
prefix's H[0] term) and ONE PE contraction with shifted weight columns:
cols 0:4 = Hp[e-1] (strict side), cols 4:8 = Hq[e] (inclusive side).
Only same-bin pairs are miscounted; each such pair's relu is < w,
giving ~1e-6 final relative error at NB=256 (validated against the
exact reference in numpy).  count stays exact (sum_{a<b} P_a Q_b from
exact class totals), as does the Q-side suffix gather QLM.

This replaces the baseline's 64 full [128, 1024] pairwise compare tiles
(~8.4M compare elements + 48K PE columns per core) with 8 [128, 256]
own-row tiles + 2 [128, 1024] gather tiles (~0.8M elements, ~4K PE
columns) plus the [8, NB] f32 AllReduce.

Sharding: each core owns a 1024-row slice of hazards/S/Y/c for both the
histogram build and the per-row gather/NLL; yc_full is replicated only
for the exact global P/Q class totals.  Final scalar is AllReduce-summed
on device, as before.
"""

import numpy as np

import concourse.mybir as mybir
import concourse.tile as tile
from concourse import bacc
from concourse.bass_utils import run_bass_kernel_spmd
from concourse.masks import make_identity

F32 = mybir.dt.float32
BF16 = mybir.dt.bfloat16
I32 = mybir.dt.int32
AF = mybir.ActivationFunctionType
ALU = mybir.AluOpType
AX = mybir.AxisListType

NCORES = 8
B, K = 8192, 4
SH = B // NCORES          # 1024 own rows (= own x-columns) per core
OWN = SH // 128           # 8 own 128-row chunks
NFULL = B // 128          # 64 chunks of the full batch (P/Q totals only)
NB = 256                  # risk bins
EBLK = NB // 128          # 2 edge partition-blocks
W = 4.0 / NB              # bin width: 2^-6, exact in bf16/f32
ALPHA = 0.15
RANKING_WEIGHT = 0.1
EPS = 1e-7

DO_COLLECTIVE = True


def _build_program():
    nc = bacc.Bacc(
        "TRN2",
        target_bir_lowering=False,
        debug=False,
        enable_asserts=False,
        num_devices=NCORES,
    )

    hz_own = nc.dram_tensor("hz_own", [SH, K], F32, kind="ExternalInput").ap()
    s_own = nc.dram_tensor("s_own", [SH, K], F32, kind="ExternalInput").ap()
    yc_own = nc.dram_tensor("yc_own", [2, SH], I32, kind="ExternalInput").ap()
    yc_full = nc.dram_tensor("yc_full", [2, B], I32, kind="ExternalInput").ap()
    out = nc.dram_tensor("out", [1, 1], F32, kind="ExternalOutput").ap()

    with tile.TileContext(nc) as tc:
        with (
            tc.tile_pool(name="const", bufs=1) as constp,
            tc.tile_pool(name="sb", bufs=1) as sb,
            tc.tile_pool(name="ps", bufs=1, space="PSUM") as ps,
            tc.tile_pool(name="pst", bufs=1, space="PSUM") as pst,
            tc.tile_pool(name="psrb", bufs=1, space="PSUM") as psrb,
            tc.tile_pool(name="dram", bufs=1, space="DRAM") as dramp,
        ):
            # ---------- input loads (criticality order; transfers serialize
            # on the DMA engines, so the risk chain's hazards go first) ------
            hzo = sb.tile([128, OWN, K], F32)
            nc.sync.dma_start(hzo[:], hz_own.rearrange("(b p) k -> p b k", p=128))
            yco = sb.tile([128, 2, OWN], I32)
            nc.sync.dma_start(yco[:], yc_own.rearrange("t (b p) -> p t b", p=128))
            yoi, coi = yco[:, 0, :], yco[:, 1, :]
            so = sb.tile([128, OWN, K], F32)
            nc.sync.dma_start(so[:], s_own.rearrange("(b p) k -> p b k", p=128))
            yc = sb.tile([128, 2, NFULL], I32)
            nc.sync.dma_start(yc[:], yc_full.rearrange("t (p b) -> p t b", p=128))
            yi, ci = yc[:, 0, :], yc[:, 1, :]

            # ---------- constants (fill the DMA-latency window) ----------
            # ACT function-table preload: dummy Ln so the load overlaps DMAs
            dumm = constp.tile([1, 1], F32)
            nc.vector.memset(dumm[:], 1.0)
            dumo = constp.tile([1, 1], F32)
            nc.scalar.activation(dumo[:], dumm[:], AF.Ln)

            ones1b = constp.tile([1, 128], BF16)
            nc.vector.memset(ones1b[:], 1.0)
            ones1 = constp.tile([1, 128], F32)
            nc.vector.memset(ones1[:], 1.0)
            onescol = constp.tile([128, 1], F32)
            nc.vector.memset(onescol[:], 1.0)
            id8 = constp.tile([8, 8], F32)
            make_identity(nc, id8[:])
            id128 = constp.tile([128, 128], F32)
            make_identity(nc, id128[:])
            # CB edges along free dim: (e+1)*w, bf16-exact
            erow_i = constp.tile([128, NB], I32)
            nc.gpsimd.iota(erow_i[:], [[1, NB]], base=1, channel_multiplier=0)
            erow = constp.tile([128, NB], BF16)
            nc.vector.tensor_scalar(erow[:], erow_i[:], W, None, op0=ALU.mult)
            # CX edges along partitions: ecol[p, blk] = (blk*128 + p)*w
            # (row 0 of block 0 is edge 0 -> an all-ones CX row)
            ecol_i = constp.tile([128, 1], I32)
            nc.gpsimd.iota(ecol_i[:], [[0, 1]], base=0, channel_multiplier=1)
            ecol = constp.tile([128, EBLK], F32)
            nc.vector.tensor_scalar(ecol[:, 0:1], ecol_i[:], W, None, op0=ALU.mult)
            nc.vector.tensor_scalar(
                ecol[:, 1:2], ecol[:, 0:1], 128.0 * W, None, op0=ALU.add
            )
            # H pad tile: col 1+t = H[t], col 0 stays 0 (memset now, no deps)
            Hpad = sb.tile([8, NB + 1], F32)
            nc.vector.memset(Hpad[:, 0:1], 0.0)

            # shared PSUM scratch bank (disjoint views; 1 bank total)
            mA = pst.tile([128, 512], F32)

            # TensorE p-state warm-up: junk matmuls spanning the input-DMA
            # window keep the PE clock at full speed for the F1/rb matmuls
            for _ in range(14):
                nc.tensor.matmul(
                    mA[:, 104:360], lhsT=ones1b[:], rhs=erow[0:1, :],
                    start=True, stop=True,
                )

            # ---------- risk chain (critical path to the histogram) ----------
            ro = sb.tile([128, OWN], F32)
            nc.vector.tensor_reduce(ro[:], hzo[:], axis=AX.X, op=ALU.add)
            rt = sb.tile([128, OWN], BF16)  # canonical r~ = bf16(r)
            nc.vector.tensor_copy(rt[:], ro[:])
            rtf = sb.tile([128, OWN], F32)  # r~ upcast (f32 scalar operand)
            nc.vector.tensor_copy(rtf[:], rt[:])

            yof = sb.tile([128, OWN], F32)
            nc.vector.tensor_copy(yof[:], yoi)
            cobar = sb.tile([128, OWN], F32)  # 1 - c_own
            nc.vector.tensor_scalar(
                cobar[:], coi, -1.0, 1.0, op0=ALU.mult, op1=ALU.add
            )

            # U_own[p, jc, u]: u 0..3 = p^a = [Y=a][1-c], u 4..7 = q^a = [Y=a]
            Uo = sb.tile([128, OWN, 8], BF16)
            tmp_eq = sb.tile([128, OWN], F32)
            for a in range(4):
                nc.vector.tensor_scalar(
                    Uo[:, :, 4 + a], yof[:], float(a), None, op0=ALU.is_equal
                )
                nc.vector.tensor_scalar(
                    tmp_eq[:], yof[:], float(a), None, op0=ALU.is_equal
                )
                nc.vector.tensor_tensor(
                    Uo[:, :, a], tmp_eq[:], cobar[:], op=ALU.mult
                )

            # rb broadcast chain (needed only by CX, well before the gather):
            # transpose r -> [8, 128], round to bf16, repack to one [1, 1024]
            # row by DMA, broadcast to all partitions via ones outer product
            ps_rT = mA[0:8, 384:512]
            nc.tensor.transpose(ps_rT, ro[:], id128[:])
            row8 = sb.tile([8, 128], BF16)
            nc.vector.tensor_copy(row8[:], ps_rT)
            row1 = sb.tile([1, SH], BF16)
            nc.sync.dma_start(row1[:], row8[:])

            # ---------- phase B: own-row cumulative histogram ----------
            # CB[i, e] = [r~_i < (e+1)w]; F1[u, e] = sum_i U[i,u] CB[i,e]
            psF = ps.tile([8, NB], F32)
            for jc in range(OWN):
                cb = sb.tile([128, NB], BF16, tag=f"cb{jc}")
                nc.vector.tensor_scalar(
                    cb[:], erow[:], rtf[:, jc : jc + 1], None, op0=ALU.is_gt
                )
                nc.tensor.matmul(
                    psF[:],
                    lhsT=Uo[:, jc, :],
                    rhs=cb[:],
                    start=(jc == 0),
                    stop=(jc == OWN - 1),
                )

            # AllReduce the partial histogram across the 8 cores
            F1s = sb.tile([8, NB], F32)
            nc.scalar.copy(F1s[:], psF[:])
            ccF_in = dramp.tile([8, NB], F32)
            ccF_out = dramp.tile([8, NB], F32)
            nc.sync.dma_start(ccF_in[:], F1s[:])
            if DO_COLLECTIVE:
                nc.gpsimd.collective_compute(
                    "AllReduce",
                    ALU.add,
                    replica_groups=[list(range(NCORES))],
                    ins=[ccF_in.opt()],
                    outs=[ccF_out.opt()],
                )
                F1g_src = ccF_out
            else:
                F1g_src = ccF_in  # timing mode: same DMA path, no collective
            F1g = sb.tile([8, NB], F32)
            nc.sync.dma_start(F1g[:], F1g_src[:])

            # rb materialization + gather compare tiles (pre-collective)
            ps_rb = psrb.tile([128, SH], F32)
            for ch in range(2):
                sl = slice(ch * 512, (ch + 1) * 512)
                nc.tensor.matmul(
                    ps_rb[:, sl], lhsT=ones1b[:], rhs=row1[0:1, sl],
                    start=True, stop=True,
                )
            rb = sb.tile([128, SH], BF16)
            nc.scalar.copy(rb[:], ps_rb[:])
            # CX[e, x] = [r~_x >= e*w]
            CX = sb.tile([128, EBLK, SH], BF16)
            for blk in range(EBLK):
                nc.vector.tensor_scalar(
                    CX[:, blk, :], rb[:], ecol[:, blk : blk + 1], None, op0=ALU.is_ge
                )

            # Y-comparison mask stacks for the fused tail reduction
            GM = sb.tile([128, OWN, 3], F32)   # [Y > a], a = 0..2
            for a in range(3):
                nc.vector.tensor_scalar(
                    GM[:, :, a], yof[:], float(a), None, op0=ALU.is_gt
                )
            LM = sb.tile([128, OWN, 3], F32)   # [Y < b], b = 1..3
            for b in range(1, 4):
                nc.vector.tensor_scalar(
                    LM[:, :, b - 1], yof[:], float(b), None, op0=ALU.is_lt
                )

            # ---------- NLL (gpsimd; overlaps everything above) ----------
            e = []
            for k in range(4):
                ek = sb.tile([128, OWN], F32, tag=f"e{k}")
                nc.gpsimd.tensor_scalar(
                    ek[:], yof[:], float(k), None, op0=ALU.is_equal
                )
                e.append(ek)
            acc = sb.tile([128, OWN], F32)

            def gather(dst, src3, shift):
                # dst = sum_k e[k] * src3[:, :, k+shift] (skipping oob)
                first = True
                for k in range(4):
                    kk = k + shift
                    if kk < 0 or kk > 3:
                        continue
                    nc.gpsimd.tensor_tensor(
                        acc[:], e[k][:], src3[:, :, kk], op=ALU.mult
                    )
                    if first:
                        nc.gpsimd.tensor_copy(dst[:], acc[:])
                        first = False
                    else:
                        nc.gpsimd.tensor_tensor(dst[:], dst[:], acc[:], op=ALU.add)

            s_now = sb.tile([128, OWN], F32)
            gather(s_now, so, 0)
            h = sb.tile([128, OWN], F32)
            gather(h, hzo, 0)
            s_prev = sb.tile([128, OWN], F32)
            gather(s_prev, so, -1)  # e1*S0 + e2*S1 + e3*S2
            nc.gpsimd.tensor_tensor(s_prev[:], s_prev[:], e[0][:], op=ALU.add)

            for t in (s_now, h, s_prev):
                nc.gpsimd.tensor_scalar(t[:], t[:], EPS, None, op0=ALU.max)

            sph = sb.tile([128, OWN], F32)
            nc.gpsimd.tensor_tensor(sph[:], s_prev[:], h[:], op=ALU.mult)

            # ---------- exact global P/Q class totals (full Y/c, gpsimd) ----
            yf = sb.tile([128, NFULL], F32)
            nc.gpsimd.tensor_copy(yf[:], yi)
            cbar = sb.tile([128, NFULL], F32)  # 1 - c
            nc.gpsimd.tensor_scalar(
                cbar[:], ci, -1.0, 1.0, op0=ALU.mult, op1=ALU.add
            )
            SS = sb.tile([128, 8], F32)
            eqf = sb.tile([128, NFULL], F32)
            pf = sb.tile([128, NFULL], F32)
            for a in range(4):
                nc.gpsimd.tensor_scalar(
                    eqf[:], yf[:], float(a), None, op0=ALU.is_equal
                )
                nc.vector.tensor_reduce(
                    SS[:, 4 + a : 5 + a], eqf[:], axis=AX.X, op=ALU.add
                )
                nc.gpsimd.tensor_tensor(pf[:], eqf[:], cbar[:], op=ALU.mult)
                nc.vector.tensor_reduce(
                    SS[:, a : a + 1], pf[:], axis=AX.X, op=ALU.add
                )
            ps_ss = mA[0:8, 0:1]
            nc.tensor.matmul(
                ps_ss, lhsT=SS[:], rhs=onescol[:], start=True, stop=True
            )
            ss_col = sb.tile([8, 1], F32)
            nc.vector.tensor_copy(ss_col[:], ps_ss)
            ps_row = mA[0:1, 8:16]
            nc.tensor.transpose(ps_row, ss_col[:], id8[:])
            pqk_row = sb.tile([1, 8], F32)  # P_0..3, Q_0..3
            nc.vector.tensor_copy(pqk_row[:], ps_row)
            ps_bc = mA[:, 32:40]
            nc.tensor.matmul(
                ps_bc, lhsT=ones1[:], rhs=pqk_row[:], start=True, stop=True
            )
            QBK = sb.tile([128, 8], F32)  # [:,0:4]=P, [:,4:8]=Q on all parts
            nc.vector.tensor_copy(QBK[:], ps_bc)

            # count = sum_{a<b} P_a Q_b; sfx[k] = sum_{b>k} Q_b
            sfx = sb.tile([128, 3], F32)
            nc.gpsimd.tensor_copy(sfx[:, 2:3], QBK[:, 7:8])
            nc.gpsimd.tensor_tensor(sfx[:, 1:2], QBK[:, 6:7], QBK[:, 7:8], op=ALU.add)
            nc.gpsimd.tensor_tensor(sfx[:, 0:1], QBK[:, 5:6], sfx[:, 1:2], op=ALU.add)
            cnt = sb.tile([128, 3], F32)
            nc.gpsimd.tensor_tensor(cnt[:], QBK[:, 0:3], sfx[:], op=ALU.mult)
            cnt1 = sb.tile([128, 1], F32)
            nc.vector.tensor_reduce(cnt1[:], cnt[:], axis=AX.X, op=ALU.add)
            rscale = sb.tile([128, 1], F32)
            nc.vector.reciprocal(rscale[:], cnt1[:])
            nc.vector.tensor_scalar(
                rscale[:], rscale[:], RANKING_WEIGHT, None, op0=ALU.mult
            )
            # QLM = sum_{b > Y_x} Q_b (exact, from the NLL one-hots)
            qa = sb.tile([128, OWN], F32)
            nc.vector.tensor_scalar(
                qa[:], e[0][:], sfx[:, 0:1], None, op0=ALU.mult
            )
            qb = sb.tile([128, OWN], F32)
            nc.vector.scalar_tensor_tensor(
                qb[:], e[1][:], sfx[:, 1:2], qa[:], op0=ALU.mult, op1=ALU.add
            )
            QLM = sb.tile([128, OWN], F32)
            nc.vector.scalar_tensor_tensor(
                QLM[:], e[2][:], sfx[:, 2:3], qb[:], op0=ALU.mult, op1=ALU.add
            )

            # ---------- NLL logs (ACT) + L assembly (gpsimd) ----------
            lnsh = sb.tile([128, OWN], F32)
            nc.scalar.activation(lnsh[:], sph[:], AF.Ln)
            lnsn = sb.tile([128, OWN], F32)
            nc.scalar.activation(lnsn[:], s_now[:], AF.Ln)
            # L = -cbar*lnsh - 0.85*lnsn + 0.85*cbar*lnsn
            Lt = sb.tile([128, OWN], F32)
            nc.gpsimd.tensor_tensor(Lt[:], cobar[:], lnsh[:], op=ALU.mult)
            t3 = sb.tile([128, OWN], F32)
            nc.gpsimd.tensor_tensor(t3[:], cobar[:], lnsn[:], op=ALU.mult)
            nc.gpsimd.tensor_scalar(
                t3[:], t3[:], 1.0 - ALPHA, None, op0=ALU.mult
            )
            nc.gpsimd.tensor_tensor(Lt[:], t3[:], Lt[:], op=ALU.subtract)
            nc.gpsimd.tensor_scalar(
                t3[:], lnsn[:], 1.0 - ALPHA, None, op0=ALU.mult
            )
            nc.gpsimd.tensor_tensor(Lt[:], Lt[:], t3[:], op=ALU.subtract)

            # ---------- post-collective: H masses + shifted weights ----------
            nc.vector.tensor_copy(Hpad[:, 1:2], F1g[:, 0:1])
            nc.vector.tensor_tensor(
                Hpad[:, 2 : NB + 1], F1g[:, 1:NB], F1g[:, 0 : NB - 1],
                op=ALU.subtract,
            )

            def ps_w(blk, half):
                o = 64 + (blk * 2 + half) * 8
                return mA[:, o : o + 8]

            for blk in range(EBLK):
                # strict side: rhs_A[e] = H[e-1]  (Hpad col offset 0)
                nc.tensor.transpose(
                    ps_w(blk, 0),
                    Hpad[0:8, blk * 128 : blk * 128 + 128],
                    id8[:],
                )
                # inclusive side: rhs_B[e] = H[e]  (Hpad col offset 1)
                nc.tensor.transpose(
                    ps_w(blk, 1),
                    Hpad[0:8, blk * 128 + 1 : blk * 128 + 129],
                    id8[:],
                )
            rhs_w = sb.tile([128, EBLK * 16], BF16)
            nc.vector.tensor_copy(rhs_w[:], mA[:, 64 : 64 + EBLK * 16])

            # ---------- V gather: V[x, jc, u] = sum_e CX[e, x] rhs_w[e, u] ----
            psV = ps.tile([128, OWN, 16], F32, tag="V")
            for jc in range(OWN):
                for blk in range(EBLK):
                    nc.tensor.matmul(
                        psV[:, jc, :],
                        lhsT=CX[:, blk, jc * 128 : (jc + 1) * 128],
                        rhs=rhs_w[:, blk * 16 : (blk + 1) * 16],
                        start=(blk == 0),
                        stop=(blk == EBLK - 1),
                    )
            Vt = sb.tile([128, OWN, 16], F32)
            nc.vector.tensor_copy(Vt[:], psV[:])

            # ---------- fused tail (single engine, last-dim reductions) ----
            # T1 = sum_a GM[.,a] * V_A[a];  T2' = sum_b LM[.,b] * V<=[b] - QLM
            TM = sb.tile([128, OWN, 3], F32)
            nc.vector.tensor_tensor(TM[:], GM[:], Vt[:, :, 0:3], op=ALU.mult)
            T1 = sb.tile([128, OWN], F32)
            nc.vector.tensor_reduce(T1[:], TM[:], axis=AX.X, op=ALU.add)
            nc.vector.tensor_tensor(TM[:], LM[:], Vt[:, :, 13:16], op=ALU.mult)
            T2 = sb.tile([128, OWN], F32)
            nc.vector.tensor_reduce(T2[:], TM[:], axis=AX.X, op=ALU.add)
            nc.vector.tensor_tensor(T2[:], T2[:], QLM[:], op=ALU.subtract)

            # contrib = r * (T1 + cbar * T2') * (0.1/count)
            contrib = sb.tile([128, OWN], F32)
            nc.vector.tensor_tensor(contrib[:], cobar[:], T2[:], op=ALU.mult)
            nc.vector.tensor_tensor(contrib[:], T1[:], contrib[:], op=ALU.add)
            rosc = sb.tile([128, OWN], F32)  # r * 0.1/count (off-path)
            nc.vector.tensor_scalar(
                rosc[:], ro[:], rscale[:, 0:1], None, op0=ALU.mult
            )
            nc.vector.tensor_tensor(contrib[:], contrib[:], rosc[:], op=ALU.mult)
            grand = sb.tile([128, OWN], F32)
            red = sb.tile([128, 1], F32)
            nc.vector.scalar_tensor_tensor(
                grand[:], Lt[:], 1.0 / B, contrib[:],
                op0=ALU.mult, op1=ALU.add, accum_out=red[:],
            )
            ps_fin = mA[0:1, 100:101]
            nc.tensor.matmul(
                ps_fin, lhsT=red[:], rhs=onescol[:], start=True, stop=True
            )

            # ---------- global sum ----------
            partial = sb.tile([1, 1], F32)
            nc.vector.tensor_copy(partial[:], ps_fin)
            if DO_COLLECTIVE:
                cc_in = dramp.tile([1, 1], F32)
                cc_out = dramp.tile([1, 1], F32)
                nc.sync.dma_start(cc_in[:], partial[:])
                nc.gpsimd.collective_compute(
                    "AllReduce",
                    ALU.add,
                    replica_groups=[list(range(NCORES))],
                    ins=[cc_in.opt()],
                    outs=[cc_out.opt()],
                )
                nc.sync.dma_start(out[:], cc_out[:])
            else:
                nc.sync.dma_start(out[:], partial[:])

    nc.compile()
    return nc


_PROGRAM = None


def _get_program():
    global _PROGRAM
    if _PROGRAM is None:
        _PROGRAM = _build_program()
    return _PROGRAM


def kernel(hazards, S, Y, c):
    hazards = np.ascontiguousarray(np.asarray(hazards, dtype=np.float32))
    S = np.ascontiguousarray(np.asarray(S, dtype=np.float32))
    Y32 = np.asarray(Y).astype(np.int32)
    c32 = np.asarray(c).astype(np.int32)
    yc_full = np.ascontiguousarray(np.stack([Y32, c32]))

    nc = _get_program()
    in_maps = []
    for m in range(NCORES):
        sl = slice(m * SH, (m + 1) * SH)
        in_maps.append(
            {
                "hz_own": np.ascontiguousarray(hazards[sl]),
                "s_own": np.ascontiguousarray(S[sl]),
                "yc_own": np.ascontiguousarray(yc_full[:, sl]),
                "yc_full": yc_full,
            }
        )
    res = run_bass_kernel_spmd(nc, in_maps, core_ids=list(range(NCORES)))
    if DO_COLLECTIVE:
        val = res.results[0]["out"][0, 0]
    else:
        val = np.float32(sum(r["out"][0, 0] for r in res.results))
    return np.asarray(val, dtype=np.float32).reshape(())


# revision 16
# speedup vs baseline: 1.7640x; 1.0263x over previous
"""CombinedSurvLoss (NLL survival + pairwise ranking) on 8 TRN2 NeuronCores.

Math
----
reference = mean_j L_j + 0.1 * total / count, where

  L_j     = -(1-c_j) * ln(clip(s_prev_j) * clip(h_j)) - 0.85 * c_j * ln(clip(s_now_j))
  total   = sum_{i,j} [c_i=0][Y_j>Y_i] relu(r_j - r_i),  r = hazards.sum(axis=1)
  count   = sum_{i,j} [c_i=0][Y_j>Y_i]

Binned-rank decomposition of the O(B^2) term
--------------------------------------------
Quantize the (bf16-rounded, canonical) risk r~ into NB uniform bins of
width w over [0, 4).  With per-class indicator weights
p^a_i = [Y_i=a][c_i=0], q^b_i = [Y_i=b] build the per-bin mass table

  H[u, t] = sum_i u_i [t*w <= r~_i < (t+1)*w]        (u over the 8 classes)

via one compare tile per own 128-row chunk (CB[i, e] = [r~_i < (e+1)w],
contracted with U_own on the TensorEngine -> cumulative F1[u, e],
AllReduced across the 8 cores as a tiny [8, NB] f32 table, then
differenced).  A pair (i, j) is counted iff bin_i < bin_j, consistently
on both sides of the decomposition

  total ~= sum_x r_x * ( sum_{a<Y_x} V_A[p^a, x]
                         - [c_x=0] sum_{b>Y_x} (Q_b - V_<=[q^b, x]) )

where V_A[u, x] = sum_t H[u,t][t <  bin_x]  (strict prefix)
      V_<=[u,x] = sum_t H[u,t][t <= bin_x]  (inclusive prefix)

Both gathers share one compare tile set CX[e, x] = [r~_x >= e*w]
(e on partitions; row e=0 is identically 1, absorbing the inclusive
prefix's H[0] term) and ONE PE contraction with shifted weight columns:
cols 0:4 = Hp[e-1] (strict side), cols 4:8 = Hq[e] (inclusive side).
Only same-bin pairs are miscounted; each such pair's relu is < w,
giving ~1e-6 final relative error at NB=256 (validated against the
exact reference in numpy).  count stays exact (sum_{a<b} P_a Q_b from
exact class totals), as does the Q-side suffix gather QLM.

This replaces the baseline's 64 full [128, 1024] pairwise compare tiles
(~8.4M compare elements + 48K PE columns per core) with 8 [128, 256]
own-row tiles + 2 [128, 1024] gather tiles (~0.8M elements, ~4K PE
columns) plus the [8, NB] f32 AllReduce.

Sharding: each core owns a 1024-row slice of hazards/S/Y/c for both the
histogram build and the per-row gather/NLL; yc_full is replicated only
for the exact global P/Q class totals.  Final scalar is AllReduce-summed
on device, as before.
"""

import numpy as np

import concourse.mybir as mybir
import concourse.tile as tile
from concourse import bacc
from concourse.bass_utils import run_bass_kernel_spmd
from concourse.masks import make_identity

F32 = mybir.dt.float32
BF16 = mybir.dt.bfloat16
I32 = mybir.dt.int32
AF = mybir.ActivationFunctionType
ALU = mybir.AluOpType
AX = mybir.AxisListType

NCORES = 8
B, K = 8192, 4
SH = B // NCORES          # 1024 own rows (= own x-columns) per core
OWN = SH // 128           # 8 own 128-row chunks
NFULL = B // 128          # 64 chunks of the full batch (P/Q totals only)
NB = 256                  # risk bins
EBLK = NB // 128          # 2 edge partition-blocks
W = 4.0 / NB              # bin width: 2^-6, exact in bf16/f32
ALPHA = 0.15
RANKING_WEIGHT = 0.1
EPS = 1e-7

DO_COLLECTIVE = True


def _build_program():
    nc = bacc.Bacc(
        "TRN2",
        target_bir_lowering=False,
        debug=False,
        enable_asserts=False,
        num_devices=NCORES,
    )

    hz_own = nc.dram_tensor("hz_own", [SH, K], F32, kind="ExternalInput").ap()
    s_own = nc.dram_tensor("s_own", [SH, K], F32, kind="ExternalInput").ap()
    yc_own = nc.dram_tensor("yc_own", [2, SH], I32, kind="ExternalInput").ap()
    yc_full = nc.dram_tensor("yc_full", [2, B], I32, kind="ExternalInput").ap()
    out = nc.dram_tensor("out", [1, 1], F32, kind="ExternalOutput").ap()

    with tile.TileContext(nc) as tc:
        with (
            tc.tile_pool(name="const", bufs=1) as constp,
            tc.tile_pool(name="sb", bufs=1) as sb,
            tc.tile_pool(name="ps", bufs=1, space="PSUM") as ps,
            tc.tile_pool(name="pst", bufs=1, space="PSUM") as pst,
            tc.tile_pool(name="psrb", bufs=1, space="PSUM") as psrb,
            tc.tile_pool(name="dram", bufs=1, space="DRAM") as dramp,
        ):
            # ---------- input loads (criticality order; transfers serialize
            # on the DMA engines, so the risk chain's hazards go first) ------
            hzo = sb.tile([128, OWN, K], F32)
            nc.sync.dma_start(hzo[:], hz_own.rearrange("(b p) k -> p b k", p=128))
            yco = sb.tile([128, 2, OWN], I32)
            nc.sync.dma_start(yco[:], yc_own.rearrange("t (b p) -> p t b", p=128))
            yoi, coi = yco[:, 0, :], yco[:, 1, :]
            so = sb.tile([128, OWN, K], F32)
            nc.sync.dma_start(so[:], s_own.rearrange("(b p) k -> p b k", p=128))
            yc = sb.tile([128, 2, NFULL], I32)
            nc.sync.dma_start(yc[:], yc_full.rearrange("t (p b) -> p t b", p=128))
            yi, ci = yc[:, 0, :], yc[:, 1, :]

            # ---------- constants (fill the DMA-latency window) ----------
            # ACT function-table preload: dummy Ln so the load overlaps DMAs
            dumm = constp.tile([1, 1], F32)
            nc.vector.memset(dumm[:], 1.0)
            dumo = constp.tile([1, 1], F32)
            nc.scalar.activation(dumo[:], dumm[:], AF.Ln)

            ones1b = constp.tile([1, 128], BF16)
            nc.vector.memset(ones1b[:], 1.0)
            ones1 = constp.tile([1, 128], F32)
            nc.vector.memset(ones1[:], 1.0)
            onescol = constp.tile([128, 1], F32)
            nc.vector.memset(onescol[:], 1.0)
            id8 = constp.tile([8, 8], F32)
            make_identity(nc, id8[:])
            id128 = constp.tile([128, 128], F32)
            make_identity(nc, id128[:])
            # CB edges along free dim: (e+1)*w, bf16-exact
            erow_i = constp.tile([128, NB], I32)
            nc.gpsimd.iota(erow_i[:], [[1, NB]], base=1, channel_multiplier=0)
            erow = constp.tile([128, NB], BF16)
            nc.vector.tensor_scalar(erow[:], erow_i[:], W, None, op0=ALU.mult)
            # CX edges along partitions: ecol[p, blk] = (blk*128 + p)*w
            # (row 0 of block 0 is edge 0 -> an all-ones CX row)
            ecol_i = constp.tile([128, 1], I32)
            nc.gpsimd.iota(ecol_i[:], [[0, 1]], base=0, channel_multiplier=1)
            ecol = constp.tile([128, EBLK], F32)
            nc.vector.tensor_scalar(ecol[:, 0:1], ecol_i[:], W, None, op0=ALU.mult)
            nc.vector.tensor_scalar(
                ecol[:, 1:2], ecol[:, 0:1], 128.0 * W, None, op0=ALU.add
            )
            # H pad tile: col 1+t = H[t], col 0 stays 0 (memset now, no deps)
            Hpad = sb.tile([8, NB + 1], F32)
            nc.vector.memset(Hpad[:, 0:1], 0.0)

            # PSUM tiles: dependency tracking is tile-granular, so each
            # latency-critical chain gets its own tile (7 banks total)
            mA = pst.tile([128, 512], F32)          # SS/QBK/final chain
            psT1 = pst.tile([8, 128], F32, tag="rT")   # r transpose
            psW = pst.tile([128, 32], F32, tag="w")    # rhs_w transposes

            # ---------- risk chain (critical path to the histogram) ----------
            ro = sb.tile([128, OWN], F32)
            nc.vector.tensor_reduce(ro[:], hzo[:], axis=AX.X, op=ALU.add)
            rt = sb.tile([128, OWN], BF16)  # canonical r~ = bf16(r)
            nc.vector.tensor_copy(rt[:], ro[:])
            rtf = sb.tile([128, OWN], F32)  # r~ upcast (f32 scalar operand)
            nc.vector.tensor_copy(rtf[:], rt[:])

            yof = sb.tile([128, OWN], F32)
            nc.vector.tensor_copy(yof[:], yoi)
            cobar = sb.tile([128, OWN], F32)  # 1 - c_own
            nc.vector.tensor_scalar(
                cobar[:], coi, -1.0, 1.0, op0=ALU.mult, op1=ALU.add
            )

            # U_own[p, jc, u]: u 0..3 = p^a = [Y=a][1-c], u 4..7 = q^a = [Y=a]
            Uo = sb.tile([128, OWN, 8], BF16)
            tmp_eq = sb.tile([128, OWN], F32)
            for a in range(4):
                nc.vector.tensor_scalar(
                    Uo[:, :, 4 + a], yof[:], float(a), None, op0=ALU.is_equal
                )
                nc.vector.tensor_scalar(
                    tmp_eq[:], yof[:], float(a), None, op0=ALU.is_equal
                )
                nc.vector.tensor_tensor(
                    Uo[:, :, a], tmp_eq[:], cobar[:], op=ALU.mult
                )

            # rb broadcast chain (needed only by CX, well before the gather):
            # transpose r -> [8, 128], round to bf16, repack to one [1, 1024]
            # row by DMA, broadcast to all partitions via ones outer product
            ps_rT = psT1[:]
            nc.tensor.transpose(ps_rT[:], ro[:], id128[:])
            row8 = sb.tile([8, 128], BF16)
            nc.vector.tensor_copy(row8[:], ps_rT)
            row1 = sb.tile([1, SH], BF16)
            nc.sync.dma_start(row1[:], row8[:])

            # ---------- phase B: own-row cumulative histogram ----------
            # CB[i, e] = [r~_i < (e+1)w]; F1[u, e] = sum_i U[i,u] CB[i,e]
            psF = ps.tile([8, NB], F32)
            # TensorE p-state warm-up: junk matmuls through the input-DMA
            # window, written into the same tiles as their real successors
            # so the tile-granular dep chains stay in program order
            for _ in range(10):
                nc.tensor.matmul(
                    psF[:], lhsT=erow[0:1, 0:8], rhs=erow[0:1, :],
                    start=True, stop=True,
                )
            for jc in range(OWN):
                cb = sb.tile([128, NB], BF16, tag=f"cb{jc}")
                nc.vector.tensor_scalar(
                    cb[:], erow[:], rtf[:, jc : jc + 1], None, op0=ALU.is_gt
                )
                nc.tensor.matmul(
                    psF[:],
                    lhsT=Uo[:, jc, :],
                    rhs=cb[:],
                    start=(jc == 0),
                    stop=(jc == OWN - 1),
                )

            # AllReduce the partial histogram across the 8 cores
            F1s = sb.tile([8, NB], F32)
            nc.scalar.copy(F1s[:], psF[:])
            ccF_in = dramp.tile([8, NB], F32)
            ccF_out = dramp.tile([8, NB], F32)
            nc.sync.dma_start(ccF_in[:], F1s[:])
            if DO_COLLECTIVE:
                nc.gpsimd.collective_compute(
                    "AllReduce",
                    ALU.add,
                    replica_groups=[list(range(NCORES))],
                    ins=[ccF_in.opt()],
                    outs=[ccF_out.opt()],
                )
                F1g_src = ccF_out
            else:
                F1g_src = ccF_in  # timing mode: same DMA path, no collective
            F1g = sb.tile([8, NB], F32)
            nc.sync.dma_start(F1g[:], F1g_src[:])

            # rb materialization + gather compare tiles (pre-collective)
            ps_rb = psrb.tile([128, SH], F32)
            for _ in range(6):
                nc.tensor.matmul(
                    ps_rb[:, 0:NB], lhsT=ones1b[:], rhs=erow[0:1, :],
                    start=True, stop=True,
                )
            for ch in range(2):
                sl = slice(ch * 512, (ch + 1) * 512)
                nc.tensor.matmul(
                    ps_rb[:, sl], lhsT=ones1b[:], rhs=row1[0:1, sl],
                    start=True, stop=True,
                )
            rb = sb.tile([128, SH], BF16)
            nc.scalar.copy(rb[:], ps_rb[:])
            # CX[e, x] = [r~_x >= e*w]
            CX = sb.tile([128, EBLK, SH], BF16)
            for blk in range(EBLK):
                nc.vector.tensor_scalar(
                    CX[:, blk, :], rb[:], ecol[:, blk : blk + 1], None, op0=ALU.is_ge
                )

            # Y-comparison mask stacks for the fused tail reduction
            GM = sb.tile([128, OWN, 3], F32)   # [Y > a], a = 0..2
            for a in range(3):
                nc.vector.tensor_scalar(
                    GM[:, :, a], yof[:], float(a), None, op0=ALU.is_gt
                )
            LM = sb.tile([128, OWN, 3], F32)   # [Y < b], b = 1..3
            for b in range(1, 4):
                nc.vector.tensor_scalar(
                    LM[:, :, b - 1], yof[:], float(b), None, op0=ALU.is_lt
                )

            # ---------- NLL (gpsimd; overlaps everything above) ----------
            e = []
            for k in range(4):
                ek = sb.tile([128, OWN], F32, tag=f"e{k}")
                nc.gpsimd.tensor_scalar(
                    ek[:], yof[:], float(k), None, op0=ALU.is_equal
                )
                e.append(ek)
            acc = sb.tile([128, OWN], F32)

            def gather(dst, src3, shift):
                # dst = sum_k e[k] * src3[:, :, k+shift] (skipping oob)
                first = True
                for k in range(4):
                    kk = k + shift
                    if kk < 0 or kk > 3:
                        continue
                    nc.gpsimd.tensor_tensor(
                        acc[:], e[k][:], src3[:, :, kk], op=ALU.mult
                    )
                    if first:
                        nc.gpsimd.tensor_copy(dst[:], acc[:])
                        first = False
                    else:
                        nc.gpsimd.tensor_tensor(dst[:], dst[:], acc[:], op=ALU.add)

            s_now = sb.tile([128, OWN], F32)
            gather(s_now, so, 0)
            h = sb.tile([128, OWN], F32)
            gather(h, hzo, 0)
            s_prev = sb.tile([128, OWN], F32)
            gather(s_prev, so, -1)  # e1*S0 + e2*S1 + e3*S2
            nc.gpsimd.tensor_tensor(s_prev[:], s_prev[:], e[0][:], op=ALU.add)

            for t in (s_now, h, s_prev):
                nc.gpsimd.tensor_scalar(t[:], t[:], EPS, None, op0=ALU.max)

            sph = sb.tile([128, OWN], F32)
            nc.gpsimd.tensor_tensor(sph[:], s_prev[:], h[:], op=ALU.mult)

            # ---------- exact global P/Q class totals (full Y/c, gpsimd) ----
            yf = sb.tile([128, NFULL], F32)
            nc.gpsimd.tensor_copy(yf[:], yi)
            cbar = sb.tile([128, NFULL], F32)  # 1 - c
            nc.gpsimd.tensor_scalar(
                cbar[:], ci, -1.0, 1.0, op0=ALU.mult, op1=ALU.add
            )
            SS = sb.tile([128, 8], F32)
            eqf = sb.tile([128, NFULL], F32)
            pf = sb.tile([128, NFULL], F32)
            for a in range(4):
                nc.gpsimd.tensor_scalar(
                    eqf[:], yf[:], float(a), None, op0=ALU.is_equal
                )
                nc.vector.tensor_reduce(
                    SS[:, 4 + a : 5 + a], eqf[:], axis=AX.X, op=ALU.add
                )
                nc.gpsimd.tensor_tensor(pf[:], eqf[:], cbar[:], op=ALU.mult)
                nc.vector.tensor_reduce(
                    SS[:, a : a + 1], pf[:], axis=AX.X, op=ALU.add
                )
            ps_ss = mA[0:8, 0:1]
            nc.tensor.matmul(
                ps_ss, lhsT=SS[:], rhs=onescol[:], start=True, stop=True
            )
            ss_col = sb.tile([8, 1], F32)
            nc.vector.tensor_copy(ss_col[:], ps_ss)
            ps_row = mA[0:1, 8:16]
            nc.tensor.transpose(ps_row, ss_col[:], id8[:])
            pqk_row = sb.tile([1, 8], F32)  # P_0..3, Q_0..3
            nc.vector.tensor_copy(pqk_row[:], ps_row)
            ps_bc = mA[:, 32:40]
            nc.tensor.matmul(
                ps_bc, lhsT=ones1[:], rhs=pqk_row[:], start=True, stop=True
            )
            QBK = sb.tile([128, 8], F32)  # [:,0:4]=P, [:,4:8]=Q on all parts
            nc.vector.tensor_copy(QBK[:], ps_bc)

            # count = sum_{a<b} P_a Q_b; sfx[k] = sum_{b>k} Q_b
            sfx = sb.tile([128, 3], F32)
            nc.gpsimd.tensor_copy(sfx[:, 2:3], QBK[:, 7:8])
            nc.gpsimd.tensor_tensor(sfx[:, 1:2], QBK[:, 6:7], QBK[:, 7:8], op=ALU.add)
            nc.gpsimd.tensor_tensor(sfx[:, 0:1], QBK[:, 5:6], sfx[:, 1:2], op=ALU.add)
            cnt = sb.tile([128, 3], F32)
            nc.gpsimd.tensor_tensor(cnt[:], QBK[:, 0:3], sfx[:], op=ALU.mult)
            cnt1 = sb.tile([128, 1], F32)
            nc.vector.tensor_reduce(cnt1[:], cnt[:], axis=AX.X, op=ALU.add)
            rscale = sb.tile([128, 1], F32)
            nc.vector.reciprocal(rscale[:], cnt1[:])
            nc.vector.tensor_scalar(
                rscale[:], rscale[:], RANKING_WEIGHT, None, op0=ALU.mult
            )
            # QLM = sum_{b > Y_x} Q_b (exact, from the NLL one-hots)
            qa = sb.tile([128, OWN], F32)
            nc.vector.tensor_scalar(
                qa[:], e[0][:], sfx[:, 0:1], None, op0=ALU.mult
            )
            qb = sb.tile([128, OWN], F32)
            nc.vector.scalar_tensor_tensor(
                qb[:], e[1][:], sfx[:, 1:2], qa[:], op0=ALU.mult, op1=ALU.add
            )
            QLM = sb.tile([128, OWN], F32)
            nc.vector.scalar_tensor_tensor(
                QLM[:], e[2][:], sfx[:, 2:3], qb[:], op0=ALU.mult, op1=ALU.add
            )

            # ---------- NLL logs (ACT) + L assembly (gpsimd) ----------
            lnsh = sb.tile([128, OWN], F32)
            nc.scalar.activation(lnsh[:], sph[:], AF.Ln)
            lnsn = sb.tile([128, OWN], F32)
            nc.scalar.activation(lnsn[:], s_now[:], AF.Ln)
            # L = -cbar*lnsh - 0.85*lnsn + 0.85*cbar*lnsn
            Lt = sb.tile([128, OWN], F32)
            nc.gpsimd.tensor_tensor(Lt[:], cobar[:], lnsh[:], op=ALU.mult)
            t3 = sb.tile([128, OWN], F32)
            nc.gpsimd.tensor_tensor(t3[:], cobar[:], lnsn[:], op=ALU.mult)
            nc.gpsimd.tensor_scalar(
                t3[:], t3[:], 1.0 - ALPHA, None, op0=ALU.mult
            )
            nc.gpsimd.tensor_tensor(Lt[:], t3[:], Lt[:], op=ALU.subtract)
            nc.gpsimd.tensor_scalar(
                t3[:], lnsn[:], 1.0 - ALPHA, None, op0=ALU.mult
            )
            nc.gpsimd.tensor_tensor(Lt[:], Lt[:], t3[:], op=ALU.subtract)

            # ---------- post-collective: H masses + shifted weights ----------
            nc.vector.tensor_copy(Hpad[:, 1:2], F1g[:, 0:1])
            nc.vector.tensor_tensor(
                Hpad[:, 2 : NB + 1], F1g[:, 1:NB], F1g[:, 0 : NB - 1],
                op=ALU.subtract,
            )

            def ps_w(blk, half):
                o = (blk * 2 + half) * 8
                return psW[:, o : o + 8]

            for blk in range(EBLK):
                # strict side: rhs_A[e] = H[e-1]  (Hpad col offset 0)
                nc.tensor.transpose(
                    ps_w(blk, 0),
                    Hpad[0:8, blk * 128 : blk * 128 + 128],
                    id8[:],
                )
                # inclusive side: rhs_B[e] = H[e]  (Hpad col offset 1)
                nc.tensor.transpose(
                    ps_w(blk, 1),
                    Hpad[0:8, blk * 128 + 1 : blk * 128 + 129],
                    id8[:],
                )
            rhs_w = sb.tile([128, EBLK * 16], BF16)
            nc.vector.tensor_copy(rhs_w[:], psW[:])

            # ---------- V gather: V[x, jc, u] = sum_e CX[e, x] rhs_w[e, u] ----
            psV = ps.tile([128, OWN, 16], F32, tag="V")
            for jc in range(OWN):
                for blk in range(EBLK):
                    nc.tensor.matmul(
                        psV[:, jc, :],
                        lhsT=CX[:, blk, jc * 128 : (jc + 1) * 128],
                        rhs=rhs_w[:, blk * 16 : (blk + 1) * 16],
                        start=(blk == 0),
                        stop=(blk == EBLK - 1),
                    )
            Vt = sb.tile([128, OWN, 16], F32)
            nc.vector.tensor_copy(Vt[:], psV[:])

            # ---------- fused tail (single engine, last-dim reductions) ----
            # T1 = sum_a GM[.,a] * V_A[a];  T2' = sum_b LM[.,b] * V<=[b] - QLM
            TM = sb.tile([128, OWN, 3], F32)
            nc.vector.tensor_tensor(TM[:], GM[:], Vt[:, :, 0:3], op=ALU.mult)
            T1 = sb.tile([128, OWN], F32)
            nc.vector.tensor_reduce(T1[:], TM[:], axis=AX.X, op=ALU.add)
            nc.vector.tensor_tensor(TM[:], LM[:], Vt[:, :, 13:16], op=ALU.mult)
            T2 = sb.tile([128, OWN], F32)
            nc.vector.tensor_reduce(T2[:], TM[:], axis=AX.X, op=ALU.add)
            nc.vector.tensor_tensor(T2[:], T2[:], QLM[:], op=ALU.subtract)

            # contrib = r * (T1 + cbar * T2') * (0.1/count)
            contrib = sb.tile([128, OWN], F32)
            nc.vector.tensor_tensor(contrib[:], cobar[:], T2[:], op=ALU.mult)
            nc.vector.tensor_tensor(contrib[:], T1[:], contrib[:], op=ALU.add)
            rosc = sb.tile([128, OWN], F32)  # r * 0.1/count (off-path)
            nc.vector.tensor_scalar(
                rosc[:], ro[:], rscale[:, 0:1], None, op0=ALU.mult
            )
            nc.vector.tensor_tensor(contrib[:], contrib[:], rosc[:], op=ALU.mult)
            grand = sb.tile([128, OWN], F32)
            red = sb.tile([128, 1], F32)
            nc.vector.scalar_tensor_tensor(
                grand[:], Lt[:], 1.0 / B, contrib[:],
                op0=ALU.mult, op1=ALU.add, accum_out=red[:],
            )
            ps_fin = mA[0:1, 100:101]
            nc.tensor.matmul(
                ps_fin, lhsT=red[:], rhs=onescol[:], start=True, stop=True
            )

            # ---------- global sum ----------
            partial = sb.tile([1, 1], F32)
            nc.vector.tensor_copy(partial[:], ps_fin)
            if DO_COLLECTIVE:
                cc_in = dramp.tile([1, 1], F32)
                cc_out = dramp.tile([1, 1], F32)
                nc.sync.dma_start(cc_in[:], partial[:])
                nc.gpsimd.collective_compute(
                    "AllReduce",
                    ALU.add,
                    replica_groups=[list(range(NCORES))],
                    ins=[cc_in.opt()],
                    outs=[cc_out.opt()],
                )
                nc.sync.dma_start(out[:], cc_out[:])
            else:
                nc.sync.dma_start(out[:], partial[:])

    nc.compile()
    return nc


_PROGRAM = None


def _get_program():
    global _PROGRAM
    if _PROGRAM is None:
        _PROGRAM = _build_program()
    return _PROGRAM


def kernel(hazards, S, Y, c):
    hazards = np.ascontiguousarray(np.asarray(hazards, dtype=np.float32))
    S = np.ascontiguousarray(np.asarray(S, dtype=np.float32))
    Y32 = np.asarray(Y).astype(np.int32)
    c32 = np.asarray(c).astype(np.int32)
    yc_full = np.ascontiguousarray(np.stack([Y32, c32]))

    nc = _get_program()
    in_maps = []
    for m in range(NCORES):
        sl = slice(m * SH, (m + 1) * SH)
        in_maps.append(
            {
                "hz_own": np.ascontiguousarray(hazards[sl]),
                "s_own": np.ascontiguousarray(S[sl]),
                "yc_own": np.ascontiguousarray(yc_full[:, sl]),
                "yc_full": yc_full,
            }
        )
    res = run_bass_kernel_spmd(nc, in_maps, core_ids=list(range(NCORES)))
    if DO_COLLECTIVE:
        val = res.results[0]["out"][0, 0]
    else:
        val = np.float32(sum(r["out"][0, 0] for r in res.results))
    return np.asarray(val, dtype=np.float32).reshape(())


# revision 19
# speedup vs baseline: 2.0125x; 1.1409x over previous
"""CombinedSurvLoss (NLL survival + pairwise ranking) on 8 TRN2 NeuronCores.

Math
----
reference = mean_j L_j + 0.1 * total / count, where

  L_j     = -(1-c_j) * ln(clip(s_prev_j) * clip(h_j)) - 0.85 * c_j * ln(clip(s_now_j))
  total   = sum_{i,j} [c_i=0][Y_j>Y_i] relu(r_j - r_i),  r = hazards.sum(axis=1)
  count   = sum_{i,j} [c_i=0][Y_j>Y_i]

Binned-rank decomposition of the O(B^2) term
--------------------------------------------
Quantize the (bf16-rounded, canonical) risk r~ into NB uniform bins of
width w over [0, 4).  With per-class indicator weights
p^a_i = [Y_i=a][c_i=0], q^b_i = [Y_i=b] build the per-bin mass table

  H[u, t] = sum_i u_i [t*w <= r~_i < (t+1)*w]        (u over the 8 classes)

via one compare tile per own 128-row chunk (CB[i, e] = [r~_i < (e+1)w],
contracted with U_own on the TensorEngine -> cumulative F1[u, e],
AllReduced across the 8 cores as a tiny [8, NB] f32 table, then
differenced).  A pair (i, j) is counted iff bin_i < bin_j, consistently
on both sides of the decomposition

  total ~= sum_x r_x * ( sum_{a<Y_x} V_A[p^a, x]
                         - [c_x=0] sum_{b>Y_x} (Q_b - V_<=[q^b, x]) )

where V_A[u, x] = sum_t H[u,t][t <  bin_x]  (strict prefix)
      V_<=[u,x] = sum_t H[u,t][t <= bin_x]  (inclusive prefix)

Both gathers share one compare tile set CX[e, x] = [r~_x >= e*w]
(e on partitions; row e=0 is identically 1, absorbing the inclusive
prefix's H[0] term) and ONE PE contraction with shifted weight columns:
cols 0:4 = Hp[e-1] (strict side), cols 4:8 = Hq[e] (inclusive side).
Only same-bin pairs are miscounted; each such pair's relu is < w,
giving ~1e-6 final relative error at NB=256 (validated against the
exact reference in numpy).  count stays exact (sum_{a<b} P_a Q_b from
exact class totals), as does the Q-side suffix gather QLM.

This replaces the baseline's 64 full [128, 1024] pairwise compare tiles
(~8.4M compare elements + 48K PE columns per core) with 8 [128, 256]
own-row tiles + 2 [128, 1024] gather tiles (~0.8M elements, ~4K PE
columns) plus the [8, NB] f32 AllReduce.

Sharding: each core owns a 1024-row slice of hazards/S/Y/c for both the
histogram build and the per-row gather/NLL; yc_full is replicated only
for the exact global P/Q class totals.  Final scalar is AllReduce-summed
on device, as before.
"""

import numpy as np

import concourse.mybir as mybir
import concourse.tile as tile
from concourse import bacc
from concourse.bass_utils import run_bass_kernel_spmd
from concourse.masks import make_identity

F32 = mybir.dt.float32
BF16 = mybir.dt.bfloat16
I32 = mybir.dt.int32
AF = mybir.ActivationFunctionType
ALU = mybir.AluOpType
AX = mybir.AxisListType

NCORES = 8
B, K = 8192, 4
SH = B // NCORES          # 1024 own rows (= own x-columns) per core
OWN = SH // 128           # 8 own 128-row chunks
NFULL = B // 128          # 64 chunks of the full batch (P/Q totals only)
NB = 256                  # risk bins
EBLK = NB // 128          # 2 edge partition-blocks
W = 4.0 / NB              # bin width: 2^-6, exact in bf16/f32
ALPHA = 0.15
RANKING_WEIGHT = 0.1
EPS = 1e-7

DO_COLLECTIVE = True


def _build_program():
    nc = bacc.Bacc(
        "TRN2",
        target_bir_lowering=False,
        debug=False,
        enable_asserts=False,
        num_devices=NCORES,
    )

    hz_own = nc.dram_tensor("hz_own", [SH, K], F32, kind="ExternalInput").ap()
    s_own = nc.dram_tensor("s_own", [SH, K], F32, kind="ExternalInput").ap()
    yc_own = nc.dram_tensor("yc_own", [2, SH], I32, kind="ExternalInput").ap()
    yc_full = nc.dram_tensor("yc_full", [2, B], I32, kind="ExternalInput").ap()
    out = nc.dram_tensor("out", [1, 1], F32, kind="ExternalOutput").ap()

    with tile.TileContext(nc) as tc:
        with (
            tc.tile_pool(name="const", bufs=1) as constp,
            tc.tile_pool(name="sb", bufs=1) as sb,
            tc.tile_pool(name="ps", bufs=1, space="PSUM") as ps,
            tc.tile_pool(name="pst", bufs=1, space="PSUM") as pst,
            tc.tile_pool(name="psrb", bufs=1, space="PSUM") as psrb,
            tc.tile_pool(name="dram", bufs=1, space="DRAM") as dramp,
        ):
            # ---------- input loads (criticality order; transfers serialize
            # on the DMA engines, so the risk chain's hazards go first) ------
            hzo = sb.tile([128, OWN, K], F32)
            nc.sync.dma_start(hzo[:], hz_own.rearrange("(b p) k -> p b k", p=128))
            yco = sb.tile([128, 2, OWN], I32)
            nc.sync.dma_start(yco[:], yc_own.rearrange("t (b p) -> p t b", p=128))
            yoi, coi = yco[:, 0, :], yco[:, 1, :]
            so = sb.tile([128, OWN, K], F32)
            nc.sync.dma_start(so[:], s_own.rearrange("(b p) k -> p b k", p=128))
            yc = sb.tile([128, 2, NFULL], I32)
            nc.sync.dma_start(yc[:], yc_full.rearrange("t (p b) -> p t b", p=128))
            yi, ci = yc[:, 0, :], yc[:, 1, :]

            # ---------- constants (fill the DMA-latency window) ----------
            # ACT function-table preload: dummy Ln so the load overlaps DMAs
            dumm = constp.tile([1, 1], F32)
            nc.vector.memset(dumm[:], 1.0)
            dumo = constp.tile([1, 1], F32)
            nc.scalar.activation(dumo[:], dumm[:], AF.Ln)

            ones1b = constp.tile([1, 128], BF16)
            nc.vector.memset(ones1b[:], 1.0)
            ones1 = constp.tile([1, 128], F32)
            nc.vector.memset(ones1[:], 1.0)
            onescol = constp.tile([128, 1], F32)
            nc.vector.memset(onescol[:], 1.0)
            id8 = constp.tile([8, 8], F32)
            make_identity(nc, id8[:])
            id128 = constp.tile([128, 128], F32)
            make_identity(nc, id128[:])
            # CB edges along free dim: (e+1)*w, bf16-exact
            erow_i = constp.tile([128, NB], I32)
            nc.gpsimd.iota(erow_i[:], [[1, NB]], base=1, channel_multiplier=0)
            erow = constp.tile([128, NB], BF16)
            nc.vector.tensor_scalar(erow[:], erow_i[:], W, None, op0=ALU.mult)
            # CX edges along partitions: ecol[p, blk] = (blk*128 + p)*w
            # (row 0 of block 0 is edge 0 -> an all-ones CX row)
            ecol_i = constp.tile([128, 1], I32)
            nc.gpsimd.iota(ecol_i[:], [[0, 1]], base=0, channel_multiplier=1)
            ecol = constp.tile([128, EBLK], F32)
            nc.vector.tensor_scalar(ecol[:, 0:1], ecol_i[:], W, None, op0=ALU.mult)
            nc.vector.tensor_scalar(
                ecol[:, 1:2], ecol[:, 0:1], 128.0 * W, None, op0=ALU.add
            )
            # H pad tile: col 1+t = H[t], col 0 stays 0 (memset now, no deps)
            Hpad = sb.tile([8, NB + 1], F32)
            nc.vector.memset(Hpad[:, 0:1], 0.0)

            # PSUM tiles: dependency tracking is tile-granular, so each
            # latency-critical chain gets its own tile (7 banks total)
            mA = pst.tile([128, 512], F32)          # SS/QBK/final chain
            psT1 = pst.tile([8, 128], F32, tag="rT")   # r transpose
            psW = pst.tile([128, 32], F32, tag="w")    # rhs_w transposes

            # ---------- risk chain (critical path to the histogram) ----------
            ro = sb.tile([128, OWN], F32)
            nc.vector.tensor_reduce(ro[:], hzo[:], axis=AX.X, op=ALU.add)
            rt = sb.tile([128, OWN], BF16)  # canonical r~ = bf16(r)
            nc.vector.tensor_copy(rt[:], ro[:])
            rtf = sb.tile([128, OWN], F32)  # r~ upcast (f32 scalar operand)
            nc.vector.tensor_copy(rtf[:], rt[:])

            yof = sb.tile([128, OWN], F32)
            nc.vector.tensor_copy(yof[:], yoi)
            cobar = sb.tile([128, OWN], F32)  # 1 - c_own
            nc.vector.tensor_scalar(
                cobar[:], coi, -1.0, 1.0, op0=ALU.mult, op1=ALU.add
            )

            # U_own[p, jc, u]: u 0..3 = p^a = [Y=a][1-c], u 4..7 = q^a = [Y=a]
            Uo = sb.tile([128, OWN, 8], BF16)
            tmp_eq = sb.tile([128, OWN], F32)
            for a in range(4):
                nc.vector.tensor_scalar(
                    Uo[:, :, 4 + a], yof[:], float(a), None, op0=ALU.is_equal
                )
                nc.vector.tensor_scalar(
                    tmp_eq[:], yof[:], float(a), None, op0=ALU.is_equal
                )
                nc.vector.tensor_tensor(
                    Uo[:, :, a], tmp_eq[:], cobar[:], op=ALU.mult
                )

            # ---------- phase B: own-row cumulative histogram ----------
            # CB[i, e] = [r~_i < (e+1)w]; F1[u, e] = sum_i U[i,u] CB[i,e]
            psF = ps.tile([8, NB], F32)
            # TensorE p-state warm-up: junk matmuls through the input-DMA
            # window, written into the same tiles as their real successors
            # so the tile-granular dep chains stay in program order
            for _ in range(10):
                nc.tensor.matmul(
                    psF[:], lhsT=erow[0:1, 0:8], rhs=erow[0:1, :],
                    start=True, stop=True,
                )
            for jc in range(OWN):
                cb = sb.tile([128, NB], BF16, tag=f"cb{jc}")
                nc.vector.tensor_scalar(
                    cb[:], erow[:], rtf[:, jc : jc + 1], None, op0=ALU.is_gt
                )
                nc.tensor.matmul(
                    psF[:],
                    lhsT=Uo[:, jc, :],
                    rhs=cb[:],
                    start=(jc == 0),
                    stop=(jc == OWN - 1),
                )

            # AllReduce the partial histogram across the 8 cores
            F1s = sb.tile([8, NB], F32)
            nc.scalar.copy(F1s[:], psF[:])
            ccF_in = dramp.tile([8, NB], F32)
            ccF_out = dramp.tile([8, NB], F32)
            nc.sync.dma_start(ccF_in[:], F1s[:])
            if DO_COLLECTIVE:
                nc.gpsimd.collective_compute(
                    "AllReduce",
                    ALU.add,
                    replica_groups=[list(range(NCORES))],
                    ins=[ccF_in.opt()],
                    outs=[ccF_out.opt()],
                )
                F1g_src = ccF_out
            else:
                F1g_src = ccF_in  # timing mode: same DMA path, no collective
            F1g = sb.tile([8, NB], F32)
            nc.sync.dma_start(F1g[:], F1g_src[:])

            # rb broadcast chain (needed only by CX, well before the gather):
            # transpose r -> [8, 128], round to bf16, repack to one [1, 1024]
            # row by DMA, broadcast to all partitions via ones outer product
            ps_rT = psT1[:]
            nc.tensor.transpose(ps_rT[:], ro[:], id128[:])
            row8 = sb.tile([8, 128], BF16)
            nc.vector.tensor_copy(row8[:], ps_rT)
            row1 = sb.tile([1, SH], BF16)
            nc.sync.dma_start(row1[:], row8[:])
            ps_rb = psrb.tile([128, SH], F32)
            for _ in range(6):
                nc.tensor.matmul(
                    ps_rb[:, 0:NB], lhsT=ones1b[:], rhs=erow[0:1, :],
                    start=True, stop=True,
                )
            for ch in range(2):
                sl = slice(ch * 512, (ch + 1) * 512)
                nc.tensor.matmul(
                    ps_rb[:, sl], lhsT=ones1b[:], rhs=row1[0:1, sl],
                    start=True, stop=True,
                )
            rb = sb.tile([128, SH], BF16)
            for ch in range(2):
                sl = slice(ch * 512, (ch + 1) * 512)
                nc.vector.tensor_copy(rb[:, sl], ps_rb[:, sl])
            # CX[e, x] = [r~_x >= e*w]
            CX = sb.tile([128, EBLK, SH], BF16)
            for blk in range(EBLK):
                nc.vector.tensor_scalar(
                    CX[:, blk, :], rb[:], ecol[:, blk : blk + 1], None, op0=ALU.is_ge
                )

            # fused-tail mask stack: cols 0:3 = [Y > a]; 3:6 = [Y < b]*(1-c)
            GLM = sb.tile([128, OWN, 6], F32)
            for a in range(3):
                nc.vector.tensor_scalar(
                    GLM[:, :, a], yof[:], float(a), None, op0=ALU.is_gt
                )
            for b in range(1, 4):
                nc.vector.tensor_scalar(
                    GLM[:, :, 2 + b], yof[:], float(b), None, op0=ALU.is_lt
                )
            for b in range(3):
                nc.vector.tensor_tensor(
                    GLM[:, :, 3 + b], GLM[:, :, 3 + b], cobar[:], op=ALU.mult
                )

            # ---------- NLL (gpsimd; overlaps everything above) ----------
            e = []
            for k in range(4):
                ek = sb.tile([128, OWN], F32, tag=f"e{k}")
                nc.gpsimd.tensor_scalar(
                    ek[:], yof[:], float(k), None, op0=ALU.is_equal
                )
                e.append(ek)
            acc = sb.tile([128, OWN], F32)

            def gather(dst, src3, shift):
                # dst = sum_k e[k] * src3[:, :, k+shift] (skipping oob)
                first = True
                for k in range(4):
                    kk = k + shift
                    if kk < 0 or kk > 3:
                        continue
                    nc.gpsimd.tensor_tensor(
                        acc[:], e[k][:], src3[:, :, kk], op=ALU.mult
                    )
                    if first:
                        nc.gpsimd.tensor_copy(dst[:], acc[:])
                        first = False
                    else:
                        nc.gpsimd.tensor_tensor(dst[:], dst[:], acc[:], op=ALU.add)

            s_now = sb.tile([128, OWN], F32)
            gather(s_now, so, 0)
            h = sb.tile([128, OWN], F32)
            gather(h, hzo, 0)
            s_prev = sb.tile([128, OWN], F32)
            gather(s_prev, so, -1)  # e1*S0 + e2*S1 + e3*S2
            nc.gpsimd.tensor_tensor(s_prev[:], s_prev[:], e[0][:], op=ALU.add)

            for t in (s_now, h, s_prev):
                nc.gpsimd.tensor_scalar(t[:], t[:], EPS, None, op0=ALU.max)

            sph = sb.tile([128, OWN], F32)
            nc.gpsimd.tensor_tensor(sph[:], s_prev[:], h[:], op=ALU.mult)

            # ---------- exact global P/Q class totals (full Y/c, gpsimd) ----
            yf = sb.tile([128, NFULL], F32)
            nc.vector.tensor_copy(yf[:], yi)
            cbar = sb.tile([128, NFULL], F32)  # 1 - c
            nc.vector.tensor_scalar(
                cbar[:], ci, -1.0, 1.0, op0=ALU.mult, op1=ALU.add
            )
            SS = sb.tile([128, 8], F32)
            eqf = sb.tile([128, NFULL], F32)
            pf = sb.tile([128, NFULL], F32)
            for a in range(4):
                nc.vector.tensor_scalar(
                    eqf[:], yf[:], float(a), None, op0=ALU.is_equal
                )
                nc.vector.tensor_reduce(
                    SS[:, 4 + a : 5 + a], eqf[:], axis=AX.X, op=ALU.add
                )
                nc.vector.tensor_tensor(pf[:], eqf[:], cbar[:], op=ALU.mult)
                nc.vector.tensor_reduce(
                    SS[:, a : a + 1], pf[:], axis=AX.X, op=ALU.add
                )
            ps_ss = mA[0:8, 0:1]
            nc.tensor.matmul(
                ps_ss, lhsT=SS[:], rhs=onescol[:], start=True, stop=True
            )
            ss_col = sb.tile([8, 1], F32)
            nc.vector.tensor_copy(ss_col[:], ps_ss)
            ps_row = mA[0:1, 8:16]
            nc.tensor.transpose(ps_row, ss_col[:], id8[:])
            pqk_row = sb.tile([1, 8], F32)  # P_0..3, Q_0..3
            nc.vector.tensor_copy(pqk_row[:], ps_row)
            ps_bc = mA[:, 32:40]
            nc.tensor.matmul(
                ps_bc, lhsT=ones1[:], rhs=pqk_row[:], start=True, stop=True
            )
            QBK = sb.tile([128, 8], F32)  # [:,0:4]=P, [:,4:8]=Q on all parts
            nc.vector.tensor_copy(QBK[:], ps_bc)

            # count = sum_{a<b} P_a Q_b; sfx[k] = sum_{b>k} Q_b
            sfx = sb.tile([128, 3], F32)
            nc.gpsimd.tensor_copy(sfx[:, 2:3], QBK[:, 7:8])
            nc.gpsimd.tensor_tensor(sfx[:, 1:2], QBK[:, 6:7], QBK[:, 7:8], op=ALU.add)
            nc.gpsimd.tensor_tensor(sfx[:, 0:1], QBK[:, 5:6], sfx[:, 1:2], op=ALU.add)
            cnt = sb.tile([128, 3], F32)
            nc.gpsimd.tensor_tensor(cnt[:], QBK[:, 0:3], sfx[:], op=ALU.mult)
            cnt1 = sb.tile([128, 1], F32)
            nc.vector.tensor_reduce(cnt1[:], cnt[:], axis=AX.X, op=ALU.add)
            rscale = sb.tile([128, 1], F32)
            nc.vector.reciprocal(rscale[:], cnt1[:])
            nc.vector.tensor_scalar(
                rscale[:], rscale[:], RANKING_WEIGHT, None, op0=ALU.mult
            )
            # QLM = sum_{b > Y_x} Q_b (exact, from the NLL one-hots)
            qa = sb.tile([128, OWN], F32)
            nc.vector.tensor_scalar(
                qa[:], e[0][:], sfx[:, 0:1], None, op0=ALU.mult
            )
            qb = sb.tile([128, OWN], F32)
            nc.vector.scalar_tensor_tensor(
                qb[:], e[1][:], sfx[:, 1:2], qa[:], op0=ALU.mult, op1=ALU.add
            )
            QLM = sb.tile([128, OWN], F32)
            nc.vector.scalar_tensor_tensor(
                QLM[:], e[2][:], sfx[:, 2:3], qb[:], op0=ALU.mult, op1=ALU.add
            )
            cQ = sb.tile([128, OWN], F32)   # cobar * QLM (off critical path)
            nc.vector.tensor_tensor(cQ[:], cobar[:], QLM[:], op=ALU.mult)
            rosc = sb.tile([128, OWN], F32)  # r * 0.1/count (off-path)
            nc.vector.tensor_scalar(
                rosc[:], ro[:], rscale[:, 0:1], None, op0=ALU.mult
            )

            # ---------- NLL logs (ACT) + L assembly (gpsimd) ----------
            lnsh = sb.tile([128, OWN], F32)
            nc.scalar.activation(lnsh[:], sph[:], AF.Ln)
            lnsn = sb.tile([128, OWN], F32)
            nc.scalar.activation(lnsn[:], s_now[:], AF.Ln)
            # L = -cbar*lnsh - 0.85*lnsn + 0.85*cbar*lnsn
            Lt = sb.tile([128, OWN], F32)
            nc.gpsimd.tensor_tensor(Lt[:], cobar[:], lnsh[:], op=ALU.mult)
            t3 = sb.tile([128, OWN], F32)
            nc.gpsimd.tensor_tensor(t3[:], cobar[:], lnsn[:], op=ALU.mult)
            nc.gpsimd.tensor_scalar(
                t3[:], t3[:], 1.0 - ALPHA, None, op0=ALU.mult
            )
            nc.gpsimd.tensor_tensor(Lt[:], t3[:], Lt[:], op=ALU.subtract)
            nc.gpsimd.tensor_scalar(
                t3[:], lnsn[:], 1.0 - ALPHA, None, op0=ALU.mult
            )
            nc.gpsimd.tensor_tensor(Lt[:], Lt[:], t3[:], op=ALU.subtract)

            # ---------- post-collective: H masses + shifted weights ----------
            nc.vector.tensor_copy(Hpad[:, 1:2], F1g[:, 0:1])
            nc.vector.tensor_tensor(
                Hpad[:, 2 : NB + 1], F1g[:, 1:NB], F1g[:, 0 : NB - 1],
                op=ALU.subtract,
            )

            def ps_w(blk, half):
                o = (blk * 2 + half) * 8
                return psW[:, o : o + 8]

            for blk in range(EBLK):
                # strict side: rhs_A[e] = H[e-1]  (Hpad col offset 0)
                nc.tensor.transpose(
                    ps_w(blk, 0),
                    Hpad[0:8, blk * 128 : blk * 128 + 128],
                    id8[:],
                )
                # inclusive side: rhs_B[e] = H[e]  (Hpad col offset 1)
                nc.tensor.transpose(
                    ps_w(blk, 1),
                    Hpad[0:8, blk * 128 + 1 : blk * 128 + 129],
                    id8[:],
                )
            rhs_w = sb.tile([128, EBLK * 16], BF16)
            nc.vector.tensor_copy(rhs_w[:], psW[:])

            # ---------- V gather: V[x, jc, u] = sum_e CX[e, x] rhs_w[e, u] ----
            psV = ps.tile([128, OWN, 16], F32, tag="V")
            for jc in range(OWN):
                for blk in range(EBLK):
                    nc.tensor.matmul(
                        psV[:, jc, :],
                        lhsT=CX[:, blk, jc * 128 : (jc + 1) * 128],
                        rhs=rhs_w[:, blk * 16 : (blk + 1) * 16],
                        start=(blk == 0),
                        stop=(blk == EBLK - 1),
                    )
            Vt2 = sb.tile([128, OWN, 6], F32)
            nc.vector.tensor_copy(Vt2[:, :, 0:3], psV[:, :, 0:3])
            nc.vector.tensor_copy(Vt2[:, :, 3:6], psV[:, :, 13:16])

            # ---------- fused tail ----------
            # contrib = (sum_a GM V_A + cbar sum_b LM V<= - cbar QLM) * rosc
            TM = sb.tile([128, OWN, 6], F32)
            contrib = sb.tile([128, OWN], F32)
            nc.vector.tensor_tensor(TM[:], GLM[:], Vt2[:], op=ALU.mult)
            nc.vector.tensor_reduce(contrib[:], TM[:], axis=AX.X, op=ALU.add)
            nc.vector.tensor_tensor(contrib[:], contrib[:], cQ[:], op=ALU.subtract)
            nc.vector.tensor_tensor(contrib[:], contrib[:], rosc[:], op=ALU.mult)
            grand = sb.tile([128, OWN], F32)
            red = sb.tile([128, 1], F32)
            nc.vector.scalar_tensor_tensor(
                grand[:], Lt[:], 1.0 / B, contrib[:],
                op0=ALU.mult, op1=ALU.add, accum_out=red[:],
            )
            ps_fin = mA[0:1, 100:101]
            nc.tensor.matmul(
                ps_fin, lhsT=red[:], rhs=onescol[:], start=True, stop=True
            )

            # ---------- global sum ----------
            partial = sb.tile([1, 1], F32)
            nc.vector.tensor_copy(partial[:], ps_fin)
            if DO_COLLECTIVE:
                cc_in = dramp.tile([1, 1], F32)
                cc_out = dramp.tile([1, 1], F32)
                nc.sync.dma_start(cc_in[:], partial[:])
                nc.gpsimd.collective_compute(
                    "AllReduce",
                    ALU.add,
                    replica_groups=[list(range(NCORES))],
                    ins=[cc_in.opt()],
                    outs=[cc_out.opt()],
                )
                nc.sync.dma_start(out[:], cc_out[:])
            else:
                nc.sync.dma_start(out[:], partial[:])

    nc.compile()
    return nc


_PROGRAM = None


def _get_program():
    global _PROGRAM
    if _PROGRAM is None:
        _PROGRAM = _build_program()
    return _PROGRAM


def kernel(hazards, S, Y, c):
    hazards = np.ascontiguousarray(np.asarray(hazards, dtype=np.float32))
    S = np.ascontiguousarray(np.asarray(S, dtype=np.float32))
    Y32 = np.asarray(Y).astype(np.int32)
    c32 = np.asarray(c).astype(np.int32)
    yc_full = np.ascontiguousarray(np.stack([Y32, c32]))

    nc = _get_program()
    in_maps = []
    for m in range(NCORES):
        sl = slice(m * SH, (m + 1) * SH)
        in_maps.append(
            {
                "hz_own": np.ascontiguousarray(hazards[sl]),
                "s_own": np.ascontiguousarray(S[sl]),
                "yc_own": np.ascontiguousarray(yc_full[:, sl]),
                "yc_full": yc_full,
            }
        )
    res = run_bass_kernel_spmd(nc, in_maps, core_ids=list(range(NCORES)))
    if DO_COLLECTIVE:
        val = res.results[0]["out"][0, 0]
    else:
        val = np.float32(sum(r["out"][0, 0] for r in res.results))
    return np.asarray(val, dtype=np.float32).reshape(())
